# revision 36
# baseline (speedup 1.0000x reference)
"""Bahdanau additive-attention kernel for Trainium2, data-parallel over
batch across 8 NeuronCores.

Per batch b:
    energy  = tanh(dec_proj[b] + enc[b] @ W_enc + b_score)   # (L, DEC)
    scores  = energy @ v                                     # (L,)
    alpha   = softmax(scores)
    att[b]  = alpha @ enc[b]                                 # (2E,)

On-device layout (per core, 8 batches):
  - enc is staged host-side TWICE, both partition-major-tiled so DMA
    descriptors are 4-16KB runs: enc8_t (fp8e4m3, chunk-major) feeds the
    PE energy matmul in perf_mode=DoubleRow (two k-subtiles per
    instruction, measured ~2x over bf16/f32r at N=512); encb_t (bf16,
    full-L tiles) feeds the DVE attention reduce. W_enc is pre-scaled by
    128 on the host so its values sit mid-range in e4m3; the tanh
    activation rescales the psum by 1/128.
  - dec_proj preamble runs in bf16 (error negligible vs fp8 energy).
  - scores = v . energyT via PE matvec in bf16 over d-tiles.
  - softmax skips the max-subtraction: |scores| <= sum|v| = 32, safely
    inside the fp32 exp range. Raw scores broadcast to 128 partitions
    with a K=1 ones-matmul; Exp runs on the broadcast tile (bf16 out)
    with accum_out giving the replicated denominator per partition.
  - att^T accumulates via fused scalar_tensor_tensor on DVE:
    accum_out[e,1] = sum_l enc_bf16[e,l] * exp_scores[l], one full-L
    instruction per (batch, k-tile) to amortize DVE fixed overhead. The
    last batch runs per-chunk so only one chunk's reduce sits in the
    kernel tail.
  - startup is DMA-paced, so batch 0 chunk 0 consumes enc in half-tile
    arrival order with the dec_proj preamble matmuls behind it.
"""

import numpy as np
import ml_dtypes
from contextlib import ExitStack

import concourse.bass as bass
import concourse.tile as tile
from concourse import mybir
from concourse.bass_utils import run_bass_kernel_spmd
from concourse.vector_clock import ScopedClock, VectorClock

N_CORES = 8
B, L, DEC, ENC2 = 64, 1024, 1024, 2048
BL = B // N_CORES  # batches per core
KT = ENC2 // 128   # contraction tiles over e
KP = KT // 2       # DoubleRow pairs over e
KH = KT // 2       # k-tiles per half (SBUF tile granularity)
DT = DEC // 128    # d tiles
LC = 512           # l-chunk (one PSUM bank of f32)
NLC = L // LC
WSCALE = 128.0     # host-side W_enc scaling for fp8 range
VSCALE = 32.0      # host-side v scaling for fp8 range

F32 = mybir.dt.float32
F32R = mybir.dt.float32r
F8 = mybir.dt.float8e4
BF16 = mybir.dt.bfloat16
AF = mybir.ActivationFunctionType
ALU = mybir.AluOpType
PM = mybir.MatmulPerfMode


def _patch_tile_drain():
    """Workarounds for this container's walrus build.

    1. The Tile tail drain carries one sem wait per touched proc; walrus
       rejects >2 on the CTRL encoding. Split the waits onto single-wait
       SP nops (SP executes in order, so the drain then needs none).
    2. Any instruction with 2+ sem waits can fail codegen (the matmul
       LW encoding holds a single wait). Split multi-wait instructions:
       excess waits move onto same-engine InstNoOp carriers inserted
       just before; engine program order makes this equivalent.
    """
    if getattr(tile.TileContext, "_drain_patched", False):
        return

    def _drain_and_barrier(self, tick_clock, wait_clock):
        vec = list(tick_clock.global_clock)
        n = len(vec)
        for i in range(n):
            if vec[i] <= 0:
                continue
            part = [0] * n
            part[i] = vec[i]
            nop_inst = self.nc.sync.nop(nofuse=True)
            wait_clock.add_sem_waits(
                nop_inst.ins, ScopedClock({None: VectorClock(part)})
            )
        self.nc.sync.drain()
        self.nc.all_engine_barrier()
        assert self.sems is not None
        popped = self.nc._tile_sem_poison_stack.pop()
        assert popped is self._sem_poison
        self.nc.clear_and_free_semaphores(list(self.sems.allocated().values()))
        self.nc.all_engine_barrier()

    tile.TileContext._drain_and_barrier = _drain_and_barrier

    import bass_rust

    orig_lower = tile.TileContext._lower_ordered_insts

    def _lower_with_wait_split(self, ordered):
        for insts in ordered.values():
            expanded = []
            for inst in insts:
                si = inst.sync_info
                waits = list(si.on_wait) if si and si.on_wait else []
                if len(waits) > 1:
                    for w in waits[:-1]:
                        nop = mybir.InstNoOp(
                            name=self.nc.get_next_instruction_name(),
                            engine=inst.engine,
                            bass_nofuse=True,
                            sync_info=bass_rust.SyncInfo(on_wait=[w], on_update=[]),
                        )
                        self.nc.register_instruction(nop)
                        expanded.append(nop)
                    inst.sync_info = bass_rust.SyncInfo(
                        on_wait=[waits[-1]],
                        on_update=list(si.on_update) if si.on_update else [],
                    )
                expanded.append(inst)
            insts[:] = expanded
        return orig_lower(self, ordered)

    tile.TileContext._lower_ordered_insts = _lower_with_wait_split
    tile.TileContext._drain_patched = True


def build_nc():
    _patch_tile_drain()
    nc = bass.Bass()
    # partition-major tiled layouts (see shard_inputs)
    enc8_t = nc.declare_dram_parameter(
        "enc8_t", [BL, NLC, 2, 128, KH, LC], F8, isOutput=False
    )
    encb_t = nc.declare_dram_parameter(
        "encb_t", [BL, 2, 128, KH, L], BF16, isOutput=False
    )
    wenc8_d = nc.declare_dram_parameter(
        "wenc8", [128, KT, DEC], F8, isOutput=False
    )
    wd_d = nc.declare_dram_parameter(
        "w_dec", [128, DT * DT * 128], BF16, isOutput=False
    )
    dec_kpb = nc.declare_dram_parameter("dec_kpb", [128, DT, BL], BF16, isOutput=False)
    b_mat = nc.declare_dram_parameter("b_mat", [128, DT], F32, isOutput=False)
    v_mat = nc.declare_dram_parameter("v_mat", [128, DT, 16], F8, isOutput=False)
    eye = nc.declare_dram_parameter("eye", [128, 128], F32, isOutput=False)
    ones = nc.declare_dram_parameter("ones", [1, 128], F32R, isOutput=False)
    att = nc.declare_dram_parameter("att", [BL, ENC2], F32, isOutput=True)

    with tile.TileContext(nc) as tc, ExitStack() as ctx:
        singles = ctx.enter_context(tc.tile_pool(name="singles", bufs=1))
        smalls = ctx.enter_context(tc.tile_pool(name="smalls", bufs=2))
        enc8_pool = ctx.enter_context(tc.tile_pool(name="enc8", bufs=8))
        encb_pool = ctx.enter_context(tc.tile_pool(name="encb", bufs=6))
        energy_pool = ctx.enter_context(tc.tile_pool(name="energy", bufs=5))
        wexp_pool = ctx.enter_context(tc.tile_pool(name="wexp", bufs=2))
        prod_pool = ctx.enter_context(tc.tile_pool(name="prod", bufs=2))
        ep_ps = ctx.enter_context(tc.tile_pool(name="ep_ps", bufs=4, space="PSUM"))
        sc_ps = ctx.enter_context(tc.tile_pool(name="sc_ps", bufs=1, space="PSUM"))
        wb_ps = ctx.enter_context(tc.tile_pool(name="wb_ps", bufs=1, space="PSUM"))
        dec_ps = ctx.enter_context(tc.tile_pool(name="dec_ps", bufs=1, space="PSUM"))
        att_ps_pool = ctx.enter_context(
            tc.tile_pool(name="att_ps", bufs=1, space="PSUM")
        )

        # ---- persistent tiles -------------------------------------------
        wenc = singles.tile([128, KT, DEC], F8)  # W_enc*128, (e-tile, k) x d
        wd_all = singles.tile([128, DT * DT, 128], BF16)  # (dt, kk) tiles
        dec_sb = singles.tile([128, DT, BL], BF16)
        b_sb = singles.tile([128, DT], F32)
        v_sb = singles.tile([128, DT, 16], F8)
        eye_sb = singles.tile([128, 128], F32)
        bias_sb = singles.tile([128, DT, BL], F32)  # dec_proj + b_score
        att_all = singles.tile([128, KT * BL], F32)  # att^T cols = b*KT+k
        ones_sb = singles.tile([1, 128], F32R)

        def wd_tile(dt, kk):
            return wd_all[:, dt * DT + kk, :]

        def alloc_chunk8(nm):
            a = enc8_pool.tile([128, KH, LC], F8, tag="enc", name=f"{nm}a")
            bb = enc8_pool.tile([128, KH, LC], F8, tag="enc", name=f"{nm}b")
            return (a, bb)

        def load_chunk8(b, c, ch):
            for h in range(2):
                nc.sync.dma_start(out=ch[h], in_=enc8_t[b, c, h])

        def alloc_encb(nm):
            a = encb_pool.tile([128, KH, L], BF16, tag="encb", name=f"{nm}a")
            bb = encb_pool.tile([128, KH, L], BF16, tag="encb", name=f"{nm}b")
            return (a, bb)

        def load_encb(b, ch):
            for h in range(2):
                nc.sync.dma_start(out=ch[h], in_=encb_t[b, h])

        def enc_pair(ch, kp):
            """[128, 2, LC] DoubleRow rhs slice for pair (2kp, 2kp+1)."""
            k = 2 * kp
            t, kk = (ch[0], k) if k < KH else (ch[1], k - KH)
            return t[:, kk : kk + 2, :]

        def encb_sl(ch, k, lo=0, width=L):
            t, kk = (ch[0], k) if k < KH else (ch[1], k - KH)
            return t[:, kk, lo : lo + width]

        def w_pair(kp, dt):
            """[128, 2, 128] DoubleRow lhsT slice."""
            return wenc[:, 2 * kp : 2 * kp + 2, dt * 128 : (dt + 1) * 128]

        # ---- startup DMA, in data-arrival order -------------------------
        def load_wd(q):
            nc.sync.dma_start(
                out=wd_all[:, 16 * q : 16 * q + 16, :],
                in_=wd_d[:, 16 * q * 128 : (16 * q + 16) * 128].rearrange(
                    "p (i m) -> p i m", m=128
                ),
            )

        enc00 = alloc_chunk8("enc00")
        nc.sync.dma_start(out=enc00[0][:, 0:2, :], in_=enc8_t[0, 0, 0][:, 0:2, :])
        nc.sync.dma_start(out=wenc[:, 0:2, :], in_=wenc8_d[:, 0:2, :])
        nc.sync.dma_start(out=enc00[0][:, 2:4, :], in_=enc8_t[0, 0, 0][:, 2:4, :])
        nc.sync.dma_start(out=wenc[:, 2:4, :], in_=wenc8_d[:, 2:4, :])
        nc.sync.dma_start(out=enc00[0][:, 4:8, :], in_=enc8_t[0, 0, 0][:, 4:8, :])
        nc.sync.dma_start(out=wenc[:, 4:8, :], in_=wenc8_d[:, 4:8, :])
        nc.sync.dma_start(out=dec_sb, in_=dec_kpb[:, :, :])
        nc.sync.dma_start(out=b_sb, in_=b_mat[:, :])
        load_wd(0)
        nc.sync.dma_start(out=enc00[1], in_=enc8_t[0, 0, 1])
        nc.sync.dma_start(out=wenc[:, 8:12, :], in_=wenc8_d[:, 8:12, :])
        nc.sync.dma_start(out=wenc[:, 12:16, :], in_=wenc8_d[:, 12:16, :])
        for q in range(1, 4):
            load_wd(q)
        nc.sync.dma_start(out=v_sb, in_=v_mat[:, :, :])
        nc.sync.dma_start(out=eye_sb, in_=eye[:, :])
        nc.sync.dma_start(out=ones_sb, in_=ones[:, :])

        # ---- chunk (0,0): consume pairs in half-arrival order, with the
        # dec_proj preamble (bf16) interleaved to fill DMA-wait gaps ----
        dpsum = dec_ps.tile([128, BL], F32)

        def emit_pre_group(dt):
            for kk in range(DT):
                nc.tensor.matmul(
                    dpsum,
                    lhsT=wd_tile(dt, kk),
                    rhs=dec_sb[:, kk, :],
                    start=(kk == 0),
                    stop=(kk == DT - 1),
                )
            nc.vector.tensor_scalar_add(
                out=bias_sb[:, dt, :], in0=dpsum, scalar1=b_sb[:, dt : dt + 1]
            )

        # preamble groups start at kp 3 so their wd-tile waits sit behind
        # the enc/wenc stream instead of stalling the first matmuls
        pre_sched = {3: [0], 4: [1, 2], 5: [3, 4], 6: [5, 6], 7: [7]}
        ps00 = {
            dt: ep_ps.tile([128, LC], F32, tag="ep", name=f"ps00_{dt}")
            for dt in range(4)
        }
        for kp in range(KP):
            for dt in range(4):
                nc.tensor.matmul(
                    ps00[dt],
                    lhsT=w_pair(kp, dt),
                    rhs=enc_pair(enc00, kp),
                    start=(kp == 0),
                    stop=(kp == KP - 1),
                    perf_mode=PM.DoubleRow,
                )
            for dt in pre_sched.get(kp, []):
                emit_pre_group(dt)

        def tanh_pair(ps, energy, b, dt):
            """tanh one d-tile's psum into half of an fp8 pair tile."""
            nc.scalar.activation(
                out=energy[:, dt % 2, :],
                in_=ps,
                func=AF.Tanh,
                bias=bias_sb[:, dt, b : b + 1],
                scale=1.0 / WSCALE,
            )

        def score_pair(sc, energy, dtp):
            """DoubleRow matvec over one d-tile pair: sc += v . energy."""
            nc.tensor.matmul(
                sc,
                lhsT=v_sb[:, 2 * dtp : 2 * dtp + 2, 0:1],
                rhs=energy,
                start=(dtp == 0),
                stop=(dtp == DT // 2 - 1),
                perf_mode=PM.DoubleRow,
            )

        def alloc_energy(nm):
            return energy_pool.tile([128, 2, LC], F8, tag="energy", name=nm)

        sc00 = sc_ps.tile([1, LC], F32, tag="sc")
        ens00 = []
        for dtp in range(2):
            en = alloc_energy(f"en00_{dtp}")
            tanh_pair(ps00[2 * dtp], en, 0, 2 * dtp)
            tanh_pair(ps00[2 * dtp + 1], en, 0, 2 * dtp + 1)
            ens00.append(en)
        for dtp in range(2, DT // 2):
            en = alloc_energy(f"en00_{dtp}")
            for i in range(2):
                dt = 2 * dtp + i
                ps = ep_ps.tile([128, LC], F32, tag="ep", name=f"ps00b_{dt}")
                for kp in range(KP):
                    nc.tensor.matmul(
                        ps,
                        lhsT=w_pair(kp, dt),
                        rhs=enc_pair(enc00, kp),
                        start=(kp == 0),
                        stop=(kp == KP - 1),
                        perf_mode=PM.DoubleRow,
                    )
                tanh_pair(ps, en, 0, dt)
            ens00.append(en)
        for dtp, en in enumerate(ens00):
            score_pair(sc00, en, dtp)

        def kmajor_chunk(b, c, enc_tile):
            """Compute one chunk's scores consuming enc pairs in DMA
            arrival order: dt 0-3 accumulate pair-major across 4 psum
            groups, then dt 4-7 run dt-major at full speed."""
            ps = {
                dt: ep_ps.tile([128, LC], F32, tag="ep", name=f"km_{b}_{c}_{dt}")
                for dt in range(4)
            }
            for kp in range(KP):
                for dt in range(4):
                    nc.tensor.matmul(
                        ps[dt],
                        lhsT=w_pair(kp, dt),
                        rhs=enc_pair(enc_tile, kp),
                        start=(kp == 0),
                        stop=(kp == KP - 1),
                        perf_mode=PM.DoubleRow,
                    )
            sc = sc_ps.tile([1, LC], F32, tag="sc", name=f"km_sc_{b}_{c}")
            ens = []
            for dtp in range(2):
                en = alloc_energy(f"enk_{b}_{c}_{dtp}")
                tanh_pair(ps[2 * dtp], en, b, 2 * dtp)
                tanh_pair(ps[2 * dtp + 1], en, b, 2 * dtp + 1)
                ens.append(en)
            for dtp in range(2, DT // 2):
                en = alloc_energy(f"enk_{b}_{c}_{dtp}")
                for i in range(2):
                    dt = 2 * dtp + i
                    p2 = ep_ps.tile(
                        [128, LC], F32, tag="ep", name=f"km2_{b}_{c}_{dt}"
                    )
                    for kp in range(KP):
                        nc.tensor.matmul(
                            p2,
                            lhsT=w_pair(kp, dt),
                            rhs=enc_pair(enc_tile, kp),
                            start=(kp == 0),
                            stop=(kp == KP - 1),
                            perf_mode=PM.DoubleRow,
                        )
                    tanh_pair(p2, en, b, dt)
                ens.append(en)
            for dtp, en in enumerate(ens):
                score_pair(sc, en, dtp)
            return sc

        def dtmajor_chunk(b, c, enc_tile):
            sc = sc_ps.tile([1, LC], F32, tag="sc", name=f"dm_sc_{b}_{c}")
            ens = []
            for dtp in range(DT // 2):
                en = alloc_energy(f"en_{b}_{c}_{dtp}")
                for i in range(2):
                    dt = 2 * dtp + i
                    ps = ep_ps.tile(
                        [128, LC], F32, tag="ep", name=f"dm_{b}_{c}_{dt}"
                    )
                    for kp in range(KP):
                        nc.tensor.matmul(
                            ps,
                            lhsT=w_pair(kp, dt),
                            rhs=enc_pair(enc_tile, kp),
                            start=(kp == 0),
                            stop=(kp == KP - 1),
                            perf_mode=PM.DoubleRow,
                        )
                    tanh_pair(ps, en, b, dt)
                ens.append(en)
            for dtp, en in enumerate(ens):
                score_pair(sc, en, dtp)
            return sc

        def score_to_wexp(sc, wexp, b, c):
            """Exp the raw chunk scores into the batch's broadcast weight
            tile; returns the per-partition denominator contribution."""
            s_sb = smalls.tile([1, LC], F32R, tag="ssb", name=f"ssb_{b}_{c}")
            nc.scalar.copy(out=s_sb, in_=sc)
            wb = wb_ps.tile([128, LC], F32, tag="wb", name=f"wb_{b}_{c}")
            nc.tensor.matmul(wb, lhsT=ones_sb, rhs=s_sb, start=True, stop=True)
            den_c = smalls.tile([128, 1], F32, tag=f"den{c}", name=f"den_{b}_{c}")
            nc.scalar.activation(
                out=wexp[:, c * LC : (c + 1) * LC],
                in_=wb,
                func=AF.Exp,
                bias=0.0,
                scale=1.0 / VSCALE,
                accum_out=den_c,
            )
            return den_c

        def batch_att(b, encb_tile, wexp):
            """Fused weighted reduce over the full L per k-tile, split
            ~3:1 between DVE (fused STT) and ACT (accum-copy over DVE
            pair-products). Emits the DVE work now; returns a closure
            with the ACT half so the caller can defer it past the next
            chunk's tanh chain (else ACT bunches and PE starves on psum
            banks)."""
            w_pairbc = bass.AP(
                tensor=wexp.tensor,
                offset=wexp.offset,
                ap=[wexp.ap[0], [0, 2], wexp.ap[1]],
            )
            pprods = []
            for kp in range(KT // 2 - 2, KT // 2):  # k 12..15 -> ACT
                k = 2 * kp
                t, kk = (encb_tile[0], k) if k < KH else (encb_tile[1], k - KH)
                prod = prod_pool.tile(
                    [128, 2, L], BF16, tag="prodp", name=f"prp_{b}_{kp}"
                )
                nc.vector.tensor_mul(out=prod, in0=t[:, kk : kk + 2, :], in1=w_pairbc)
                pprods.append((k, prod))
            for k in range(KT - 4):
                col = b * KT + k
                prod = prod_pool.tile([128, L], BF16, tag="prod", name=f"pr_{b}_{k}")
                nc.vector.scalar_tensor_tensor(
                    out=prod,
                    in0=encb_sl(encb_tile, k),
                    scalar=1.0,
                    in1=wexp,
                    op0=ALU.mult,
                    op1=ALU.mult,
                    accum_out=att_all[:, col : col + 1],
                )

            def act_half():
                scr = smalls.tile([128, L], BF16, tag="ascr", name=f"ascr_b{b}")
                for k, prod in pprods:
                    for i in range(2):
                        col = b * KT + k + i
                        nc.scalar.activation(
                            out=scr,
                            in_=prod[:, i, :],
                            func=AF.Copy,
                            bias=0.0,
                            scale=1.0,
                            accum_out=att_all[:, col : col + 1],
                        )

            return act_half

        def chunk_att(b, c, encb_tile, wexp):
            """Per-chunk variant (used for the last batch to keep the
            kernel tail to one chunk's reduce)."""
            atmp = None
            if c > 0:
                atmp = smalls.tile([128, KT], F32, tag="atmp", name=f"atmp_{b}_{c}")
            for k in range(KT):
                col = b * KT + k
                prod = prod_pool.tile(
                    [128, L], BF16, tag="prod", name=f"prc_{b}_{c}_{k}"
                )
                dst = att_all[:, col : col + 1] if c == 0 else atmp[:, k : k + 1]
                nc.vector.scalar_tensor_tensor(
                    out=prod[:, 0:LC],
                    in0=encb_sl(encb_tile, k, c * LC, LC),
                    scalar=1.0,
                    in1=wexp[:, c * LC : (c + 1) * LC],
                    op0=ALU.mult,
                    op1=ALU.mult,
                    accum_out=dst,
                )
            if c > 0:
                cols = slice(b * KT, (b + 1) * KT)
                nc.vector.tensor_add(
                    out=att_all[:, cols], in0=att_all[:, cols], in1=atmp
                )

        def chunk_att_tail(b, c, encb_tile, wexp):
            """Kernel-tail variant: split the reduce between DVE (fused
            STT, k 0-7) and ACT (accum-copy over DVE pair-products,
            k 8-15) so the exposed tail is ~2/3 shorter."""
            atmp = None
            if c > 0:
                atmp = smalls.tile([128, KT], F32, tag="atmp", name=f"atmpt_{b}_{c}")

            def dst(k):
                if c == 0:
                    col = b * KT + k
                    return att_all[:, col : col + 1]
                return atmp[:, k : k + 1]

            w_sl = wexp[:, c * LC : (c + 1) * LC]
            w_pairbc = bass.AP(
                tensor=w_sl.tensor,
                offset=w_sl.offset,
                ap=[w_sl.ap[0], [0, 2], w_sl.ap[1]],
            )
            pprods = []
            for kp in range(KT // 4, KT // 2):
                k = 2 * kp
                t, kk = (encb_tile[0], k) if k < KH else (encb_tile[1], k - KH)
                prod = prod_pool.tile(
                    [128, 2, LC], BF16, tag="prodt", name=f"prp_{b}_{c}_{kp}"
                )
                nc.vector.tensor_mul(
                    out=prod,
                    in0=t[:, kk : kk + 2, c * LC : (c + 1) * LC],
                    in1=w_pairbc,
                )
                pprods.append((k, prod))
            for k in range(KT // 2):
                prod = prod_pool.tile(
                    [128, L], BF16, tag="prod", name=f"prt_{b}_{c}_{k}"
                )
                nc.vector.scalar_tensor_tensor(
                    out=prod[:, 0:LC],
                    in0=encb_sl(encb_tile, k, c * LC, LC),
                    scalar=1.0,
                    in1=w_sl,
                    op0=ALU.mult,
                    op1=ALU.mult,
                    accum_out=dst(k),
                )

            def act_half():
                scr = smalls.tile([128, LC], BF16, tag="ascr", name=f"ascr_{b}_{c}")
                for k, prod in pprods:
                    for i in range(2):
                        nc.scalar.activation(
                            out=scr,
                            in_=prod[:, i, :],
                            func=AF.Copy,
                            bias=0.0,
                            scale=1.0,
                            accum_out=dst(k + i),
                        )
                if c > 0:
                    cols = slice(b * KT, (b + 1) * KT)
                    nc.vector.tensor_add(
                        out=att_all[:, cols], in0=att_all[:, cols], in1=atmp
                    )

            return act_half

        def batch_epilogue(b, dens):
            """Transpose the raw attention columns and store, folding the
            softmax normalization into the ACT psum->sbuf copy (scale)."""
            rden = smalls.tile([128, 1], F32, tag="rden")
            nc.vector.tensor_add(out=rden, in0=dens[0], in1=dens[1])
            for extra in dens[2:]:
                nc.vector.tensor_add(out=rden, in0=rden, in1=extra)
            nc.vector.reciprocal(out=rden, in_=rden)
            cols = slice(b * KT, (b + 1) * KT)
            att_bt = att_ps_pool.tile([KT, 128], F32, tag="abt")
            nc.tensor.transpose(att_bt, att_all[:, cols], eye_sb)
            att_sb = smalls.tile([KT, 128], F32, tag="asb")
            nc.scalar.activation(
                out=att_sb,
                in_=att_bt,
                func=AF.Copy,
                bias=0.0,
                scale=rden[0:KT, :],
            )
            nc.sync.dma_start(
                out=att[b].rearrange("(k p) -> k p", p=128), in_=att_sb
            )

        # ---- main loop (chunk (0,0) scores already computed above) ------
        wexp_b = wexp_pool.tile([128, L], BF16, tag="wexp", name="wexp_0")
        dens = [score_to_wexp(sc00, wexp_b, 0, 0)]
        encb_b = None
        encb_next = None
        pending_act = []
        for b in range(BL):
            last_b = b == BL - 1
            for c in range(NLC):
                if (b, c) == (0, 0):
                    continue
                enc_tile = alloc_chunk8(f"enc_{b}_{c}")
                load_chunk8(b, c, enc_tile)
                if c == 0:
                    wexp_b = wexp_pool.tile(
                        [128, L], BF16, tag="wexp", name=f"wexp_{b}"
                    )
                    if b == 1:
                        # batch 1's bf16 copy loads behind chunk (1,0)
                        encb_next = alloc_encb("encb_1")
                        load_encb(1, encb_next)
                    encb_b = encb_next
                else:
                    if b == 0:
                        # batch 0's bf16 copy loads behind chunk (0,1)
                        encb_b = alloc_encb("encb_0")
                        load_encb(0, encb_b)
                    if 1 <= b < BL - 1:
                        # prefetch next batch's bf16 copy a chunk early
                        encb_next = alloc_encb(f"encb_{b + 1}")
                        load_encb(b + 1, encb_next)
                if (b, c) in ((0, 1), (1, 0), (1, 1)):
                    sc = kmajor_chunk(b, c, enc_tile)
                else:
                    sc = dtmajor_chunk(b, c, enc_tile)
                dens.append(score_to_wexp(sc, wexp_b, b, c))
                # deferred ACT halves of earlier reduces go here, where
                # ACT has inter-chunk slack (after this chunk's scores)
                for fn in pending_act:
                    fn()
                pending_act = []
                if c == 1 and b > 0:
                    # deferred epilogue: PE transpose of batch b-1 sits
                    # a full chunk behind its DVE reduce, so it never
                    # stalls PE waiting on the normalize
                    batch_epilogue(b - 1, prev_dens)
                if b >= BL - 2:
                    # last two batches: per-chunk DVE/ACT split spreads
                    # the late reduce load and keeps the tail short
                    pending_act.append(chunk_att_tail(b, c, encb_b, wexp_b))
            if b < BL - 2:
                pending_act.append(batch_att(b, encb_b, wexp_b))
            prev_dens = dens
            dens = []
        for fn in pending_act:
            fn()
        batch_epilogue(BL - 1, prev_dens)

    return nc


def shard_inputs(dec_hidden, enc_output, W_score, b_score, v):
    """Full inputs -> per-core input maps (host-side layout staging)."""
    dec_hidden = np.ascontiguousarray(dec_hidden, dtype=np.float32)
    W_score = np.asarray(W_score, dtype=np.float32)
    # W_dec tiled partition-major: [p, dt, kk, m] with value W[kk*128+p, dt*128+m]
    wd_t = np.ascontiguousarray(
        W_score[:DEC]
        .reshape(DT, 128, DT, 128)
        .transpose(1, 2, 0, 3)
        .reshape(128, DT * DT * 128)
    ).astype(ml_dtypes.bfloat16)
    # W_enc tiled partition-major: [p, k, d]
    wenc8 = np.ascontiguousarray(
        (W_score[DEC:] * WSCALE).reshape(KT, 128, DEC).transpose(1, 0, 2)
    ).astype(ml_dtypes.float8_e4m3)
    b_mat = np.ascontiguousarray(
        np.asarray(b_score, dtype=np.float32).reshape(DT, 128).T
    )
    v_pd = np.asarray(v, dtype=np.float32).reshape(DT, 128).T * VSCALE
    v_mat = np.zeros((128, DT, 16), dtype=ml_dtypes.float8_e4m3)
    v_mat[:, :, 0] = v_pd.astype(ml_dtypes.float8_e4m3)
    eye = np.eye(128, dtype=np.float32)

    in_maps = []
    for core in range(N_CORES):
        sl = slice(core * BL, (core + 1) * BL)
        # (L, BL, 2E) -> (BL, 2E, L)
        enc_t = np.ascontiguousarray(
            np.asarray(enc_output[:, sl, :], dtype=np.float32).transpose(1, 2, 0)
        )
        # fp8 chunk-major partition-tiled: [b, c, half, p, k, l]
        enc8_t = np.ascontiguousarray(
            enc_t.reshape(BL, 2, KH, 128, NLC, LC).transpose(0, 4, 1, 3, 2, 5)
        ).astype(ml_dtypes.float8_e4m3)
        # bf16 full-L partition-tiled: [b, half, p, k, l]
        encb_t = np.ascontiguousarray(
            enc_t.reshape(BL, 2, KH, 128, L).transpose(0, 1, 3, 2, 4)
        ).astype(ml_dtypes.bfloat16)
        # (BL, DEC) -> [p, kt, b]
        dec_kpb = (
            np.ascontiguousarray(
                dec_hidden[sl].T.reshape(DT, 128, BL).transpose(1, 0, 2)
            )
            .astype(ml_dtypes.bfloat16)
        )
        in_maps.append(
            {
                "enc8_t": enc8_t,
                "encb_t": encb_t,
                "ones": np.ones((1, 128), dtype=np.float32),
                "dec_kpb": dec_kpb,
                "wenc8": wenc8,
                "w_dec": wd_t,
                "b_mat": b_mat,
                "v_mat": v_mat,
                "eye": eye,
            }
        )
    return in_maps


_NC_CACHE = None


def kernel(dec_hidden, enc_output, W_score, b_score, v):
    global _NC_CACHE
    if _NC_CACHE is None:
        _NC_CACHE = build_nc()
    nc = _NC_CACHE
    in_maps = shard_inputs(dec_hidden, enc_output, W_score, b_score, v)
    res = run_bass_kernel_spmd(nc, in_maps, list(range(N_CORES)))
    return np.concatenate([res.results[i]["att"] for i in range(N_CORES)], axis=0)


# revision 37
# speedup vs baseline: 1.1776x; 1.1776x over previous
"""Bahdanau additive-attention kernel for Trainium2, data-parallel over
batch across 8 NeuronCores.

Per batch b:
    energy  = tanh(dec_proj[b] + enc[b] @ W_enc + b_score)   # (L, DEC)
    scores  = energy @ v                                     # (L,)
    alpha   = softmax(scores)
    att[b]  = alpha @ enc[b]                                 # (2E,)

On-device layout (per core, 8 batches):
  - enc is staged host-side TWICE, both partition-major-tiled so DMA
    descriptors are 4-16KB runs: enc8_t (fp8e4m3, chunk-major) feeds the
    PE energy matmul in perf_mode=DoubleRow (two k-subtiles per
    instruction, measured ~2x over bf16/f32r at N=512); encb_t (bf16,
    full-L tiles) feeds the DVE attention reduce. W_enc is pre-scaled by
    128 on the host so its values sit mid-range in e4m3; the tanh
    activation rescales the psum by 1/128.
  - dec_proj preamble runs in bf16 (error negligible vs fp8 energy).
  - scores = v . energyT via PE matvec in bf16 over d-tiles.
  - softmax skips the max-subtraction: |scores| <= sum|v| = 32, safely
    inside the fp32 exp range. Raw scores broadcast to 128 partitions
    with a K=1 ones-matmul; Exp runs on the broadcast tile (bf16 out)
    with accum_out giving the replicated denominator per partition.
  - att^T accumulates via fused scalar_tensor_tensor on DVE:
    accum_out[e,1] = sum_l enc_bf16[e,l] * exp_scores[l], one full-L
    instruction per (batch, k-tile) to amortize DVE fixed overhead. The
    last batch runs per-chunk so only one chunk's reduce sits in the
    kernel tail.
  - startup is DMA-paced, so batch 0 chunk 0 consumes enc in half-tile
    arrival order with the dec_proj preamble matmuls behind it.
"""

import numpy as np
import ml_dtypes
from contextlib import ExitStack

import concourse.bass as bass
import concourse.tile as tile
from concourse import mybir
from concourse.bass_utils import run_bass_kernel_spmd
from concourse.vector_clock import ScopedClock, VectorClock

N_CORES = 8
B, L, DEC, ENC2 = 64, 1024, 1024, 2048
BL = B // N_CORES  # batches per core
KT = ENC2 // 128   # contraction tiles over e
KP = KT // 2       # DoubleRow pairs over e
KH = KT // 2       # k-tiles per half (SBUF tile granularity)
DT = DEC // 128    # d tiles
LC = 512           # l-chunk (one PSUM bank of f32)
NLC = L // LC
WSCALE = 128.0     # host-side W_enc scaling for fp8 range
VSCALE = 32.0      # host-side v scaling for fp8 range

F32 = mybir.dt.float32
F32R = mybir.dt.float32r
F8 = mybir.dt.float8e4
BF16 = mybir.dt.bfloat16
AF = mybir.ActivationFunctionType
ALU = mybir.AluOpType
PM = mybir.MatmulPerfMode


def _patch_tile_drain():
    """Workarounds for this container's walrus build.

    1. The Tile tail drain carries one sem wait per touched proc; walrus
       rejects >2 on the CTRL encoding. Split the waits onto single-wait
       SP nops (SP executes in order, so the drain then needs none).
    2. Any instruction with 2+ sem waits can fail codegen (the matmul
       LW encoding holds a single wait). Split multi-wait instructions:
       excess waits move onto same-engine InstNoOp carriers inserted
       just before; engine program order makes this equivalent.
    """
    if getattr(tile.TileContext, "_drain_patched", False):
        return

    def _drain_and_barrier(self, tick_clock, wait_clock):
        vec = list(tick_clock.global_clock)
        n = len(vec)
        for i in range(n):
            if vec[i] <= 0:
                continue
            part = [0] * n
            part[i] = vec[i]
            nop_inst = self.nc.sync.nop(nofuse=True)
            wait_clock.add_sem_waits(
                nop_inst.ins, ScopedClock({None: VectorClock(part)})
            )
        self.nc.sync.drain()
        self.nc.all_engine_barrier()
        assert self.sems is not None
        popped = self.nc._tile_sem_poison_stack.pop()
        assert popped is self._sem_poison
        self.nc.clear_and_free_semaphores(list(self.sems.allocated().values()))
        self.nc.all_engine_barrier()

    tile.TileContext._drain_and_barrier = _drain_and_barrier

    import bass_rust

    orig_lower = tile.TileContext._lower_ordered_insts

    def _lower_with_wait_split(self, ordered):
        for insts in ordered.values():
            expanded = []
            for inst in insts:
                si = inst.sync_info
                waits = list(si.on_wait) if si and si.on_wait else []
                if len(waits) > 1:
                    for w in waits[:-1]:
                        nop = mybir.InstNoOp(
                            name=self.nc.get_next_instruction_name(),
                            engine=inst.engine,
                            bass_nofuse=True,
                            sync_info=bass_rust.SyncInfo(on_wait=[w], on_update=[]),
                        )
                        self.nc.register_instruction(nop)
                        expanded.append(nop)
                    inst.sync_info = bass_rust.SyncInfo(
                        on_wait=[waits[-1]],
                        on_update=list(si.on_update) if si.on_update else [],
                    )
                expanded.append(inst)
            insts[:] = expanded
        return orig_lower(self, ordered)

    tile.TileContext._lower_ordered_insts = _lower_with_wait_split
    tile.TileContext._drain_patched = True


def build_nc():
    _patch_tile_drain()
    nc = bass.Bass()
    # partition-major tiled layouts (see shard_inputs)
    enc8_t = nc.declare_dram_parameter(
        "enc8_t", [BL, NLC, 2, 128, KH, LC], F8, isOutput=False
    )
    encb_t = nc.declare_dram_parameter(
        "encb_t", [BL, 2, 128, KH, L], BF16, isOutput=False
    )
    wenc8_d = nc.declare_dram_parameter(
        "wenc8", [128, KT, DEC], F8, isOutput=False
    )
    wd_d = nc.declare_dram_parameter(
        "w_dec", [128, DT * DT * 128], BF16, isOutput=False
    )
    dec_kpb = nc.declare_dram_parameter("dec_kpb", [128, DT, BL], BF16, isOutput=False)
    b_mat = nc.declare_dram_parameter("b_mat", [128, DT], F32, isOutput=False)
    v_mat = nc.declare_dram_parameter("v_mat", [128, DT, 16], F8, isOutput=False)
    eye = nc.declare_dram_parameter("eye", [128, 128], F32, isOutput=False)
    ones = nc.declare_dram_parameter("ones", [1, 128], F32R, isOutput=False)
    att = nc.declare_dram_parameter("att", [BL, ENC2], F32, isOutput=True)

    with tile.TileContext(nc) as tc, ExitStack() as ctx:
        singles = ctx.enter_context(tc.tile_pool(name="singles", bufs=1))
        smalls = ctx.enter_context(tc.tile_pool(name="smalls", bufs=2))
        enc8_pool = ctx.enter_context(tc.tile_pool(name="enc8", bufs=8))
        encb_pool = ctx.enter_context(tc.tile_pool(name="encb", bufs=6))
        energy_pool = ctx.enter_context(tc.tile_pool(name="energy", bufs=3))
        wexp_pool = ctx.enter_context(tc.tile_pool(name="wexp", bufs=2))
        prod_pool = ctx.enter_context(tc.tile_pool(name="prod", bufs=2))
        ep_ps = ctx.enter_context(tc.tile_pool(name="ep_ps", bufs=4, space="PSUM"))
        sc_ps = ctx.enter_context(tc.tile_pool(name="sc_ps", bufs=1, space="PSUM"))
        wb_ps = ctx.enter_context(tc.tile_pool(name="wb_ps", bufs=1, space="PSUM"))
        dec_ps = ctx.enter_context(tc.tile_pool(name="dec_ps", bufs=1, space="PSUM"))
        att_ps_pool = ctx.enter_context(
            tc.tile_pool(name="att_ps", bufs=1, space="PSUM")
        )

        # ---- persistent tiles -------------------------------------------
        wenc = singles.tile([128, KT, DEC], F8)  # W_enc*128, (e-tile, k) x d
        wd_all = singles.tile([128, DT * DT, 128], BF16)  # (dt, kk) tiles
        dec_sb = singles.tile([128, DT, BL], BF16)
        b_sb = singles.tile([128, DT], F32)
        v_sb = singles.tile([128, DT, 16], F8)
        eye_sb = singles.tile([128, 128], F32)
        bias_sb = singles.tile([128, DT, BL], F32)  # dec_proj + b_score
        att_all = singles.tile([128, KT * BL], F32)  # att^T cols = b*KT+k
        ones_sb = singles.tile([1, 128], F32R)

        def wd_tile(dt, kk):
            return wd_all[:, dt * DT + kk, :]

        def alloc_chunk8(nm):
            a = enc8_pool.tile([128, KH, LC], F8, tag="enc", name=f"{nm}a")
            bb = enc8_pool.tile([128, KH, LC], F8, tag="enc", name=f"{nm}b")
            return (a, bb)

        def load_chunk8(b, c, ch):
            for h in range(2):
                nc.sync.dma_start(out=ch[h], in_=enc8_t[b, c, h])

        def alloc_encb(nm):
            a = encb_pool.tile([128, KH, L], BF16, tag="encb", name=f"{nm}a")
            bb = encb_pool.tile([128, KH, L], BF16, tag="encb", name=f"{nm}b")
            return (a, bb)

        def load_encb(b, ch):
            for h in range(2):
                nc.sync.dma_start(out=ch[h], in_=encb_t[b, h])

        def enc_pair(ch, kp):
            """[128, 2, LC] DoubleRow rhs slice for pair (2kp, 2kp+1)."""
            k = 2 * kp
            t, kk = (ch[0], k) if k < KH else (ch[1], k - KH)
            return t[:, kk : kk + 2, :]

        def encb_sl(ch, k, lo=0, width=L):
            t, kk = (ch[0], k) if k < KH else (ch[1], k - KH)
            return t[:, kk, lo : lo + width]

        def w_pair(kp, dt):
            """[128, 2, 128] DoubleRow lhsT slice."""
            return wenc[:, 2 * kp : 2 * kp + 2, dt * 128 : (dt + 1) * 128]

        # ---- startup DMA, in data-arrival order -------------------------
        def load_wd(q):
            nc.sync.dma_start(
                out=wd_all[:, 16 * q : 16 * q + 16, :],
                in_=wd_d[:, 16 * q * 128 : (16 * q + 16) * 128].rearrange(
                    "p (i m) -> p i m", m=128
                ),
            )

        enc00 = alloc_chunk8("enc00")
        nc.sync.dma_start(out=enc00[0][:, 0:2, :], in_=enc8_t[0, 0, 0][:, 0:2, :])
        nc.sync.dma_start(out=wenc[:, 0:2, :], in_=wenc8_d[:, 0:2, :])
        nc.sync.dma_start(out=enc00[0][:, 2:4, :], in_=enc8_t[0, 0, 0][:, 2:4, :])
        nc.sync.dma_start(out=wenc[:, 2:4, :], in_=wenc8_d[:, 2:4, :])
        nc.sync.dma_start(out=enc00[0][:, 4:8, :], in_=enc8_t[0, 0, 0][:, 4:8, :])
        nc.sync.dma_start(out=wenc[:, 4:8, :], in_=wenc8_d[:, 4:8, :])
        nc.sync.dma_start(out=dec_sb, in_=dec_kpb[:, :, :])
        nc.sync.dma_start(out=b_sb, in_=b_mat[:, :])
        load_wd(0)
        nc.sync.dma_start(out=enc00[1], in_=enc8_t[0, 0, 1])
        nc.sync.dma_start(out=wenc[:, 8:12, :], in_=wenc8_d[:, 8:12, :])
        nc.sync.dma_start(out=wenc[:, 12:16, :], in_=wenc8_d[:, 12:16, :])
        for q in range(1, 4):
            load_wd(q)
        nc.sync.dma_start(out=v_sb, in_=v_mat[:, :, :])
        nc.sync.dma_start(out=eye_sb, in_=eye[:, :])
        nc.sync.dma_start(out=ones_sb, in_=ones[:, :])

        # ---- chunk (0,0): consume pairs in half-arrival order, with the
        # dec_proj preamble (bf16) interleaved to fill DMA-wait gaps ----
        dpsum = dec_ps.tile([128, BL], F32)

        def emit_pre_group(dt):
            for kk in range(DT):
                nc.tensor.matmul(
                    dpsum,
                    lhsT=wd_tile(dt, kk),
                    rhs=dec_sb[:, kk, :],
                    start=(kk == 0),
                    stop=(kk == DT - 1),
                )
            nc.vector.tensor_scalar_add(
                out=bias_sb[:, dt, :], in0=dpsum, scalar1=b_sb[:, dt : dt + 1]
            )

        # preamble groups start at kp 3 so their wd-tile waits sit behind
        # the enc/wenc stream instead of stalling the first matmuls
        pre_sched = {3: [0], 4: [1, 2], 5: [3, 4], 6: [5, 6], 7: [7]}
        ps00 = {
            dt: ep_ps.tile([128, LC], F32, tag="ep", name=f"ps00_{dt}")
            for dt in range(4)
        }
        for kp in range(KP):
            for dt in range(4):
                nc.tensor.matmul(
                    ps00[dt],
                    lhsT=w_pair(kp, dt),
                    rhs=enc_pair(enc00, kp),
                    start=(kp == 0),
                    stop=(kp == KP - 1),
                    perf_mode=PM.DoubleRow,
                )
            for dt in pre_sched.get(kp, []):
                emit_pre_group(dt)

        def tanh_pair(ps, energy, b, dt):
            """tanh one d-tile's psum into half of an fp8 pair tile."""
            nc.scalar.activation(
                out=energy[:, dt % 2, :],
                in_=ps,
                func=AF.Tanh,
                bias=bias_sb[:, dt, b : b + 1],
                scale=1.0 / WSCALE,
            )

        def score_pair(sc, energy, dtp):
            """DoubleRow matvec over one d-tile pair: sc += v . energy."""
            nc.tensor.matmul(
                sc,
                lhsT=v_sb[:, 2 * dtp : 2 * dtp + 2, 0:1],
                rhs=energy,
                start=(dtp == 0),
                stop=(dtp == DT // 2 - 1),
                perf_mode=PM.DoubleRow,
            )

        def alloc_energy(nm):
            return energy_pool.tile([128, 2, LC], F8, tag="energy", name=nm)

        sc00 = sc_ps.tile([1, LC], F32, tag="sc")
        for dtp in range(2):
            en = alloc_energy(f"en00_{dtp}")
            tanh_pair(ps00[2 * dtp], en, 0, 2 * dtp)
            tanh_pair(ps00[2 * dtp + 1], en, 0, 2 * dtp + 1)
            score_pair(sc00, en, dtp)
        for dtp in range(2, DT // 2):
            en = alloc_energy(f"en00_{dtp}")
            for i in range(2):
                dt = 2 * dtp + i
                ps = ep_ps.tile([128, LC], F32, tag="ep", name=f"ps00b_{dt}")
                for kp in range(KP):
                    nc.tensor.matmul(
                        ps,
                        lhsT=w_pair(kp, dt),
                        rhs=enc_pair(enc00, kp),
                        start=(kp == 0),
                        stop=(kp == KP - 1),
                        perf_mode=PM.DoubleRow,
                    )
                tanh_pair(ps, en, 0, dt)
            score_pair(sc00, en, dtp)

        def kmajor_chunk(b, c, enc_tile):
            """Compute one chunk's scores consuming enc pairs in DMA
            arrival order: dt 0-3 accumulate pair-major across 4 psum
            groups, then dt 4-7 run dt-major at full speed."""
            ps = {
                dt: ep_ps.tile([128, LC], F32, tag="ep", name=f"km_{b}_{c}_{dt}")
                for dt in range(4)
            }
            for kp in range(KP):
                for dt in range(4):
                    nc.tensor.matmul(
                        ps[dt],
                        lhsT=w_pair(kp, dt),
                        rhs=enc_pair(enc_tile, kp),
                        start=(kp == 0),
                        stop=(kp == KP - 1),
                        perf_mode=PM.DoubleRow,
                    )
            sc = sc_ps.tile([1, LC], F32, tag="sc", name=f"km_sc_{b}_{c}")
            for dtp in range(2):
                en = alloc_energy(f"enk_{b}_{c}_{dtp}")
                tanh_pair(ps[2 * dtp], en, b, 2 * dtp)
                tanh_pair(ps[2 * dtp + 1], en, b, 2 * dtp + 1)
                score_pair(sc, en, dtp)
            for dtp in range(2, DT // 2):
                en = alloc_energy(f"enk_{b}_{c}_{dtp}")
                for i in range(2):
                    dt = 2 * dtp + i
                    p2 = ep_ps.tile(
                        [128, LC], F32, tag="ep", name=f"km2_{b}_{c}_{dt}"
                    )
                    for kp in range(KP):
                        nc.tensor.matmul(
                            p2,
                            lhsT=w_pair(kp, dt),
                            rhs=enc_pair(enc_tile, kp),
                            start=(kp == 0),
                            stop=(kp == KP - 1),
                            perf_mode=PM.DoubleRow,
                        )
                    tanh_pair(p2, en, b, dt)
                score_pair(sc, en, dtp)
            return sc

        def dtmajor_chunk(b, c, enc_tile):
            sc = sc_ps.tile([1, LC], F32, tag="sc", name=f"dm_sc_{b}_{c}")
            for dtp in range(DT // 2):
                en = alloc_energy(f"en_{b}_{c}_{dtp}")
                for i in range(2):
                    dt = 2 * dtp + i
                    ps = ep_ps.tile(
                        [128, LC], F32, tag="ep", name=f"dm_{b}_{c}_{dt}"
                    )
                    for kp in range(KP):
                        nc.tensor.matmul(
                            ps,
                            lhsT=w_pair(kp, dt),
                            rhs=enc_pair(enc_tile, kp),
                            start=(kp == 0),
                            stop=(kp == KP - 1),
                            perf_mode=PM.DoubleRow,
                        )
                    tanh_pair(ps, en, b, dt)
                score_pair(sc, en, dtp)
            return sc

        def score_to_wexp(sc, wexp, b, c):
            """Exp the raw chunk scores into the batch's broadcast weight
            tile; returns the per-partition denominator contribution."""
            s_sb = smalls.tile([1, LC], F32R, tag="ssb", name=f"ssb_{b}_{c}")
            nc.scalar.copy(out=s_sb, in_=sc)
            wb = wb_ps.tile([128, LC], F32, tag="wb", name=f"wb_{b}_{c}")
            nc.tensor.matmul(wb, lhsT=ones_sb, rhs=s_sb, start=True, stop=True)
            den_c = smalls.tile([128, 1], F32, tag=f"den{c}", name=f"den_{b}_{c}")
            nc.scalar.activation(
                out=wexp[:, c * LC : (c + 1) * LC],
                in_=wb,
                func=AF.Exp,
                bias=0.0,
                scale=1.0 / VSCALE,
                accum_out=den_c,
            )
            return den_c

        def batch_att(b, encb_tile, wexp):
            """Fused weighted reduce over the full L per k-tile, split
            ~3:1 between DVE (fused STT) and ACT (accum-copy over DVE
            pair-products). Emits the DVE work now; returns a closure
            with the ACT half so the caller can defer it past the next
            chunk's tanh chain (else ACT bunches and PE starves on psum
            banks)."""
            w_pairbc = bass.AP(
                tensor=wexp.tensor,
                offset=wexp.offset,
                ap=[wexp.ap[0], [0, 2], wexp.ap[1]],
            )
            pprods = []
            for kp in range(KT // 2 - 1, KT // 2):  # k 14..15 -> ACT
                k = 2 * kp
                t, kk = (encb_tile[0], k) if k < KH else (encb_tile[1], k - KH)
                prod = prod_pool.tile(
                    [128, 2, L], BF16, tag="prodp", name=f"prp_{b}_{kp}"
                )
                nc.vector.tensor_mul(out=prod, in0=t[:, kk : kk + 2, :], in1=w_pairbc)
                pprods.append((k, prod))
            for k in range(KT - 2):
                col = b * KT + k
                prod = prod_pool.tile([128, L], BF16, tag="prod", name=f"pr_{b}_{k}")
                nc.vector.scalar_tensor_tensor(
                    out=prod,
                    in0=encb_sl(encb_tile, k),
                    scalar=1.0,
                    in1=wexp,
                    op0=ALU.mult,
                    op1=ALU.mult,
                    accum_out=att_all[:, col : col + 1],
                )

            def act_half():
                scr = smalls.tile([128, L], BF16, tag="ascr", name=f"ascr_b{b}")
                for k, prod in pprods:
                    for i in range(2):
                        col = b * KT + k + i
                        nc.scalar.activation(
                            out=scr,
                            in_=prod[:, i, :],
                            func=AF.Copy,
                            bias=0.0,
                            scale=1.0,
                            accum_out=att_all[:, col : col + 1],
                        )

            return act_half

        def chunk_att(b, c, encb_tile, wexp):
            """Per-chunk variant (used for the last batch to keep the
            kernel tail to one chunk's reduce)."""
            atmp = None
            if c > 0:
                atmp = smalls.tile([128, KT], F32, tag="atmp", name=f"atmp_{b}_{c}")
            for k in range(KT):
                col = b * KT + k
                prod = prod_pool.tile(
                    [128, L], BF16, tag="prod", name=f"prc_{b}_{c}_{k}"
                )
                dst = att_all[:, col : col + 1] if c == 0 else atmp[:, k : k + 1]
                nc.vector.scalar_tensor_tensor(
                    out=prod[:, 0:LC],
                    in0=encb_sl(encb_tile, k, c * LC, LC),
                    scalar=1.0,
                    in1=wexp[:, c * LC : (c + 1) * LC],
                    op0=ALU.mult,
                    op1=ALU.mult,
                    accum_out=dst,
                )
            if c > 0:
                cols = slice(b * KT, (b + 1) * KT)
                nc.vector.tensor_add(
                    out=att_all[:, cols], in0=att_all[:, cols], in1=atmp
                )

        def chunk_att_tail(b, c, encb_tile, wexp):
            """Kernel-tail variant: split the reduce between DVE (fused
            STT, k 0-7) and ACT (accum-copy over DVE pair-products,
            k 8-15) so the exposed tail is ~2/3 shorter."""
            atmp = None
            if c > 0:
                atmp = smalls.tile([128, KT], F32, tag="atmp", name=f"atmpt_{b}_{c}")

            def dst(k):
                if c == 0:
                    col = b * KT + k
                    return att_all[:, col : col + 1]
                return atmp[:, k : k + 1]

            w_sl = wexp[:, c * LC : (c + 1) * LC]
            w_pairbc = bass.AP(
                tensor=w_sl.tensor,
                offset=w_sl.offset,
                ap=[w_sl.ap[0], [0, 2], w_sl.ap[1]],
            )
            pprods = []
            for kp in range(KT // 4, KT // 2):
                k = 2 * kp
                t, kk = (encb_tile[0], k) if k < KH else (encb_tile[1], k - KH)
                prod = prod_pool.tile(
                    [128, 2, LC], BF16, tag="prodt", name=f"prp_{b}_{c}_{kp}"
                )
                nc.vector.tensor_mul(
                    out=prod,
                    in0=t[:, kk : kk + 2, c * LC : (c + 1) * LC],
                    in1=w_pairbc,
                )
                pprods.append((k, prod))
            for k in range(KT // 2):
                prod = prod_pool.tile(
                    [128, L], BF16, tag="prod", name=f"prt_{b}_{c}_{k}"
                )
                nc.vector.scalar_tensor_tensor(
                    out=prod[:, 0:LC],
                    in0=encb_sl(encb_tile, k, c * LC, LC),
                    scalar=1.0,
                    in1=w_sl,
                    op0=ALU.mult,
                    op1=ALU.mult,
                    accum_out=dst(k),
                )

            def act_half():
                scr = smalls.tile([128, LC], BF16, tag="ascr", name=f"ascr_{b}_{c}")
                for k, prod in pprods:
                    for i in range(2):
                        nc.scalar.activation(
                            out=scr,
                            in_=prod[:, i, :],
                            func=AF.Copy,
                            bias=0.0,
                            scale=1.0,
                            accum_out=dst(k + i),
                        )
                if c > 0:
                    cols = slice(b * KT, (b + 1) * KT)
                    nc.vector.tensor_add(
                        out=att_all[:, cols], in0=att_all[:, cols], in1=atmp
                    )

            return act_half

        def batch_epilogue(b, dens):
            """Transpose the raw attention columns and store, folding the
            softmax normalization into the ACT psum->sbuf copy (scale)."""
            rden = smalls.tile([128, 1], F32, tag="rden")
            nc.vector.tensor_add(out=rden, in0=dens[0], in1=dens[1])
            for extra in dens[2:]:
                nc.vector.tensor_add(out=rden, in0=rden, in1=extra)
            nc.vector.reciprocal(out=rden, in_=rden)
            cols = slice(b * KT, (b + 1) * KT)
            att_bt = att_ps_pool.tile([KT, 128], F32, tag="abt")
            nc.tensor.transpose(att_bt, att_all[:, cols], eye_sb)
            att_sb = smalls.tile([KT, 128], F32, tag="asb")
            nc.scalar.activation(
                out=att_sb,
                in_=att_bt,
                func=AF.Copy,
                bias=0.0,
                scale=rden[0:KT, :],
            )
            nc.sync.dma_start(
                out=att[b].rearrange("(k p) -> k p", p=128), in_=att_sb
            )

        # ---- main loop (chunk (0,0) scores already computed above) ------
        wexp_b = wexp_pool.tile([128, L], BF16, tag="wexp", name="wexp_0")
        dens = [score_to_wexp(sc00, wexp_b, 0, 0)]
        encb_b = None
        encb_next = None
        pending_act = []
        for b in range(BL):
            last_b = b == BL - 1
            for c in range(NLC):
                if (b, c) == (0, 0):
                    continue
                enc_tile = alloc_chunk8(f"enc_{b}_{c}")
                load_chunk8(b, c, enc_tile)
                if c == 0:
                    wexp_b = wexp_pool.tile(
                        [128, L], BF16, tag="wexp", name=f"wexp_{b}"
                    )
                    if b == 1:
                        # batch 1's bf16 copy loads behind chunk (1,0)
                        encb_next = alloc_encb("encb_1")
                        load_encb(1, encb_next)
                    encb_b = encb_next
                else:
                    if b == 0:
                        # batch 0's bf16 copy loads behind chunk (0,1)
                        encb_b = alloc_encb("encb_0")
                        load_encb(0, encb_b)
                    if 1 <= b < BL - 1:
                        # prefetch next batch's bf16 copy a chunk early
                        encb_next = alloc_encb(f"encb_{b + 1}")
                        load_encb(b + 1, encb_next)
                if (b, c) in ((0, 1), (1, 0), (1, 1)):
                    sc = kmajor_chunk(b, c, enc_tile)
                else:
                    sc = dtmajor_chunk(b, c, enc_tile)
                dens.append(score_to_wexp(sc, wexp_b, b, c))
                # deferred ACT halves of earlier reduces go here, where
                # ACT has inter-chunk slack (after this chunk's scores)
                for fn in pending_act:
                    fn()
                pending_act = []
                if c == 1 and b > 0:
                    # deferred epilogue: PE transpose of batch b-1 sits
                    # a full chunk behind its DVE reduce, so it never
                    # stalls PE waiting on the normalize
                    batch_epilogue(b - 1, prev_dens)
                if b >= BL - 2:
                    # last two batches: per-chunk DVE/ACT split spreads
                    # the late reduce load and keeps the tail short
                    pending_act.append(chunk_att_tail(b, c, encb_b, wexp_b))
            if b < BL - 2:
                pending_act.append(batch_att(b, encb_b, wexp_b))
            prev_dens = dens
            dens = []
        for fn in pending_act:
            fn()
        batch_epilogue(BL - 1, prev_dens)

    return nc


def shard_inputs(dec_hidden, enc_output, W_score, b_score, v):
    """Full inputs -> per-core input maps (host-side layout staging)."""
    dec_hidden = np.ascontiguousarray(dec_hidden, dtype=np.float32)
    W_score = np.asarray(W_score, dtype=np.float32)
    # W_dec tiled partition-major: [p, dt, kk, m] with value W[kk*128+p, dt*128+m]
    wd_t = np.ascontiguousarray(
        W_score[:DEC]
        .reshape(DT, 128, DT, 128)
        .transpose(1, 2, 0, 3)
        .reshape(128, DT * DT * 128)
    ).astype(ml_dtypes.bfloat16)
    # W_enc tiled partition-major: [p, k, d]
    wenc8 = np.ascontiguousarray(
        (W_score[DEC:] * WSCALE).reshape(KT, 128, DEC).transpose(1, 0, 2)
    ).astype(ml_dtypes.float8_e4m3)
    b_mat = np.ascontiguousarray(
        np.asarray(b_score, dtype=np.float32).reshape(DT, 128).T
    )
    v_pd = np.asarray(v, dtype=np.float32).reshape(DT, 128).T * VSCALE
    v_mat = np.zeros((128, DT, 16), dtype=ml_dtypes.float8_e4m3)
    v_mat[:, :, 0] = v_pd.astype(ml_dtypes.float8_e4m3)
    eye = np.eye(128, dtype=np.float32)

    in_maps = []
    for core in range(N_CORES):
        sl = slice(core * BL, (core + 1) * BL)
        # (L, BL, 2E) -> (BL, 2E, L)
        enc_t = np.ascontiguousarray(
            np.asarray(enc_output[:, sl, :], dtype=np.float32).transpose(1, 2, 0)
        )
        # fp8 chunk-major partition-tiled: [b, c, half, p, k, l]
        enc8_t = np.ascontiguousarray(
            enc_t.reshape(BL, 2, KH, 128, NLC, LC).transpose(0, 4, 1, 3, 2, 5)
        ).astype(ml_dtypes.float8_e4m3)
        # bf16 full-L partition-tiled: [b, half, p, k, l]
        encb_t = np.ascontiguousarray(
            enc_t.reshape(BL, 2, KH, 128, L).transpose(0, 1, 3, 2, 4)
        ).astype(ml_dtypes.bfloat16)
        # (BL, DEC) -> [p, kt, b]
        dec_kpb = (
            np.ascontiguousarray(
                dec_hidden[sl].T.reshape(DT, 128, BL).transpose(1, 0, 2)
            )
            .astype(ml_dtypes.bfloat16)
        )
        in_maps.append(
            {
                "enc8_t": enc8_t,
                "encb_t": encb_t,
                "ones": np.ones((1, 128), dtype=np.float32),
                "dec_kpb": dec_kpb,
                "wenc8": wenc8,
                "w_dec": wd_t,
                "b_mat": b_mat,
                "v_mat": v_mat,
                "eye": eye,
            }
        )
    return in_maps


_NC_CACHE = None


def kernel(dec_hidden, enc_output, W_score, b_score, v):
    global _NC_CACHE
    if _NC_CACHE is None:
        _NC_CACHE = build_nc()
    nc = _NC_CACHE
    in_maps = shard_inputs(dec_hidden, enc_output, W_score, b_score, v)
    res = run_bass_kernel_spmd(nc, in_maps, list(range(N_CORES)))
    return np.concatenate([res.results[i]["att"] for i in range(N_CORES)], axis=0)


# revision 38
# speedup vs baseline: 1.1801x; 1.0021x over previous
"""Bahdanau additive-attention kernel for Trainium2, data-parallel over
batch across 8 NeuronCores.

Per batch b:
    energy  = tanh(dec_proj[b] + enc[b] @ W_enc + b_score)   # (L, DEC)
    scores  = energy @ v                                     # (L,)
    alpha   = softmax(scores)
    att[b]  = alpha @ enc[b]                                 # (2E,)

On-device layout (per core, 8 batches):
  - enc is staged host-side TWICE, both partition-major-tiled so DMA
    descriptors are 4-16KB runs: enc8_t (fp8e4m3, chunk-major) feeds the
    PE energy matmul in perf_mode=DoubleRow (two k-subtiles per
    instruction, measured ~2x over bf16/f32r at N=512); encb_t (bf16,
    full-L tiles) feeds the DVE attention reduce. W_enc is pre-scaled by
    128 on the host so its values sit mid-range in e4m3; the tanh
    activation rescales the psum by 1/128.
  - dec_proj preamble runs in bf16 (error negligible vs fp8 energy).
  - scores = v . energyT via PE matvec in bf16 over d-tiles.
  - softmax skips the max-subtraction: |scores| <= sum|v| = 32, safely
    inside the fp32 exp range. Raw scores broadcast to 128 partitions
    with a K=1 ones-matmul; Exp runs on the broadcast tile (bf16 out)
    with accum_out giving the replicated denominator per partition.
  - att^T accumulates via fused scalar_tensor_tensor on DVE:
    accum_out[e,1] = sum_l enc_bf16[e,l] * exp_scores[l], one full-L
    instruction per (batch, k-tile) to amortize DVE fixed overhead. The
    last batch runs per-chunk so only one chunk's reduce sits in the
    kernel tail.
  - startup is DMA-paced, so batch 0 chunk 0 consumes enc in half-tile
    arrival order with the dec_proj preamble matmuls behind it.
"""

import numpy as np
import ml_dtypes
from contextlib import ExitStack

import concourse.bass as bass
import concourse.tile as tile
from concourse import mybir
from concourse.bass_utils import run_bass_kernel_spmd
from concourse.vector_clock import ScopedClock, VectorClock

N_CORES = 8
B, L, DEC, ENC2 = 64, 1024, 1024, 2048
BL = B // N_CORES  # batches per core
KT = ENC2 // 128   # contraction tiles over e
KP = KT // 2       # DoubleRow pairs over e
KH = KT // 2       # k-tiles per half (SBUF tile granularity)
DT = DEC // 128    # d tiles
LC = 512           # l-chunk (one PSUM bank of f32)
NLC = L // LC
WSCALE = 128.0     # host-side W_enc scaling for fp8 range
VSCALE = 32.0      # host-side v scaling for fp8 range

F32 = mybir.dt.float32
F32R = mybir.dt.float32r
F8 = mybir.dt.float8e4
BF16 = mybir.dt.bfloat16
AF = mybir.ActivationFunctionType
ALU = mybir.AluOpType
PM = mybir.MatmulPerfMode


def _patch_tile_drain():
    """Workarounds for this container's walrus build.

    1. The Tile tail drain carries one sem wait per touched proc; walrus
       rejects >2 on the CTRL encoding. Split the waits onto single-wait
       SP nops (SP executes in order, so the drain then needs none).
    2. Any instruction with 2+ sem waits can fail codegen (the matmul
       LW encoding holds a single wait). Split multi-wait instructions:
       excess waits move onto same-engine InstNoOp carriers inserted
       just before; engine program order makes this equivalent.
    """
    if getattr(tile.TileContext, "_drain_patched", False):
        return

    def _drain_and_barrier(self, tick_clock, wait_clock):
        vec = list(tick_clock.global_clock)
        n = len(vec)
        for i in range(n):
            if vec[i] <= 0:
                continue
            part = [0] * n
            part[i] = vec[i]
            nop_inst = self.nc.sync.nop(nofuse=True)
            wait_clock.add_sem_waits(
                nop_inst.ins, ScopedClock({None: VectorClock(part)})
            )
        self.nc.sync.drain()
        self.nc.all_engine_barrier()
        assert self.sems is not None
        popped = self.nc._tile_sem_poison_stack.pop()
        assert popped is self._sem_poison
        self.nc.clear_and_free_semaphores(list(self.sems.allocated().values()))
        self.nc.all_engine_barrier()

    tile.TileContext._drain_and_barrier = _drain_and_barrier

    import bass_rust

    orig_lower = tile.TileContext._lower_ordered_insts

    def _lower_with_wait_split(self, ordered):
        for insts in ordered.values():
            expanded = []
            for inst in insts:
                si = inst.sync_info
                waits = list(si.on_wait) if si and si.on_wait else []
                if len(waits) > 1:
                    for w in waits[:-1]:
                        nop = mybir.InstNoOp(
                            name=self.nc.get_next_instruction_name(),
                            engine=inst.engine,
                            bass_nofuse=True,
                            sync_info=bass_rust.SyncInfo(on_wait=[w], on_update=[]),
                        )
                        self.nc.register_instruction(nop)
                        expanded.append(nop)
                    inst.sync_info = bass_rust.SyncInfo(
                        on_wait=[waits[-1]],
                        on_update=list(si.on_update) if si.on_update else [],
                    )
                expanded.append(inst)
            insts[:] = expanded
        return orig_lower(self, ordered)

    tile.TileContext._lower_ordered_insts = _lower_with_wait_split
    tile.TileContext._drain_patched = True


def build_nc():
    _patch_tile_drain()
    nc = bass.Bass()
    # partition-major tiled layouts (see shard_inputs)
    enc8_t = nc.declare_dram_parameter(
        "enc8_t", [BL, NLC, 2, 128, KH, LC], F8, isOutput=False
    )
    encb_t = nc.declare_dram_parameter(
        "encb_t", [BL, 2, 128, KH, L], BF16, isOutput=False
    )
    wenc8_d = nc.declare_dram_parameter(
        "wenc8", [128, KT, DEC], F8, isOutput=False
    )
    bias_d = nc.declare_dram_parameter("bias_kpb", [128, DT, BL], F32, isOutput=False)
    v_mat = nc.declare_dram_parameter("v_mat", [128, DT, 16], F8, isOutput=False)
    eye = nc.declare_dram_parameter("eye", [128, 128], F32, isOutput=False)
    ones = nc.declare_dram_parameter("ones", [1, 128], F32R, isOutput=False)
    att = nc.declare_dram_parameter("att", [BL, ENC2], F32, isOutput=True)

    with tile.TileContext(nc) as tc, ExitStack() as ctx:
        singles = ctx.enter_context(tc.tile_pool(name="singles", bufs=1))
        smalls = ctx.enter_context(tc.tile_pool(name="smalls", bufs=2))
        enc8_pool = ctx.enter_context(tc.tile_pool(name="enc8", bufs=8))
        encb_pool = ctx.enter_context(tc.tile_pool(name="encb", bufs=6))
        energy_pool = ctx.enter_context(tc.tile_pool(name="energy", bufs=3))
        wexp_pool = ctx.enter_context(tc.tile_pool(name="wexp", bufs=2))
        prod_pool = ctx.enter_context(tc.tile_pool(name="prod", bufs=2))
        ep_ps = ctx.enter_context(tc.tile_pool(name="ep_ps", bufs=4, space="PSUM"))
        sc_ps = ctx.enter_context(tc.tile_pool(name="sc_ps", bufs=2, space="PSUM"))
        wb_ps = ctx.enter_context(tc.tile_pool(name="wb_ps", bufs=1, space="PSUM"))
        att_ps_pool = ctx.enter_context(
            tc.tile_pool(name="att_ps", bufs=1, space="PSUM")
        )

        # ---- persistent tiles -------------------------------------------
        wenc = singles.tile([128, KT, DEC], F8)  # W_enc*128, (e-tile, k) x d
        v_sb = singles.tile([128, DT, 16], F8)
        eye_sb = singles.tile([128, 128], F32)
        bias_sb = singles.tile([128, DT, BL], F32)  # dec_proj + b_score
        att_all = singles.tile([128, KT * BL], F32)  # att^T cols = b*KT+k
        ones_sb = singles.tile([1, 128], F32R)

        def alloc_chunk8(nm):
            a = enc8_pool.tile([128, KH, LC], F8, tag="enc", name=f"{nm}a")
            bb = enc8_pool.tile([128, KH, LC], F8, tag="enc", name=f"{nm}b")
            return (a, bb)

        def load_chunk8(b, c, ch):
            for h in range(2):
                nc.sync.dma_start(out=ch[h], in_=enc8_t[b, c, h])

        def alloc_encb(nm):
            a = encb_pool.tile([128, KH, L], BF16, tag="encb", name=f"{nm}a")
            bb = encb_pool.tile([128, KH, L], BF16, tag="encb", name=f"{nm}b")
            return (a, bb)

        def load_encb(b, ch):
            for h in range(2):
                nc.sync.dma_start(out=ch[h], in_=encb_t[b, h])

        def enc_pair(ch, kp):
            """[128, 2, LC] DoubleRow rhs slice for pair (2kp, 2kp+1)."""
            k = 2 * kp
            t, kk = (ch[0], k) if k < KH else (ch[1], k - KH)
            return t[:, kk : kk + 2, :]

        def encb_sl(ch, k, lo=0, width=L):
            t, kk = (ch[0], k) if k < KH else (ch[1], k - KH)
            return t[:, kk, lo : lo + width]

        def w_pair(kp, dt):
            """[128, 2, 128] DoubleRow lhsT slice."""
            return wenc[:, 2 * kp : 2 * kp + 2, dt * 128 : (dt + 1) * 128]

        # ---- startup DMA, in data-arrival order -------------------------
        enc00 = alloc_chunk8("enc00")
        nc.sync.dma_start(out=enc00[0][:, 0:2, :], in_=enc8_t[0, 0, 0][:, 0:2, :])
        nc.sync.dma_start(out=wenc[:, 0:2, :], in_=wenc8_d[:, 0:2, :])
        nc.sync.dma_start(out=enc00[0][:, 2:4, :], in_=enc8_t[0, 0, 0][:, 2:4, :])
        nc.sync.dma_start(out=wenc[:, 2:4, :], in_=wenc8_d[:, 2:4, :])
        nc.sync.dma_start(out=enc00[0][:, 4:8, :], in_=enc8_t[0, 0, 0][:, 4:8, :])
        nc.sync.dma_start(out=wenc[:, 4:8, :], in_=wenc8_d[:, 4:8, :])
        nc.sync.dma_start(out=bias_sb, in_=bias_d[:, :, :])
        nc.sync.dma_start(out=v_sb, in_=v_mat[:, :, :])
        nc.sync.dma_start(out=enc00[1], in_=enc8_t[0, 0, 1])
        nc.sync.dma_start(out=wenc[:, 8:12, :], in_=wenc8_d[:, 8:12, :])
        nc.sync.dma_start(out=wenc[:, 12:16, :], in_=wenc8_d[:, 12:16, :])
        nc.sync.dma_start(out=eye_sb, in_=eye[:, :])
        nc.sync.dma_start(out=ones_sb, in_=ones[:, :])

        # ---- chunk (0,0): consume pairs in half-arrival order ----------
        ps00 = {
            dt: ep_ps.tile([128, LC], F32, tag="ep", name=f"ps00_{dt}")
            for dt in range(4)
        }
        for kp in range(KP):
            for dt in range(4):
                nc.tensor.matmul(
                    ps00[dt],
                    lhsT=w_pair(kp, dt),
                    rhs=enc_pair(enc00, kp),
                    start=(kp == 0),
                    stop=(kp == KP - 1),
                    perf_mode=PM.DoubleRow,
                )

        def tanh_pair(ps, energy, b, dt):
            """tanh one d-tile's psum into half of an fp8 pair tile."""
            nc.scalar.activation(
                out=energy[:, dt % 2, :],
                in_=ps,
                func=AF.Tanh,
                bias=bias_sb[:, dt, b : b + 1],
                scale=1.0 / WSCALE,
            )

        def score_pair(sc, energy, dtp):
            """DoubleRow matvec over one d-tile pair: sc += v . energy."""
            nc.tensor.matmul(
                sc,
                lhsT=v_sb[:, 2 * dtp : 2 * dtp + 2, 0:1],
                rhs=energy,
                start=(dtp == 0),
                stop=(dtp == DT // 2 - 1),
                perf_mode=PM.DoubleRow,
            )

        def alloc_energy(nm):
            return energy_pool.tile([128, 2, LC], F8, tag="energy", name=nm)

        sc00 = sc_ps.tile([1, LC], F32, tag="sc")
        for dtp in range(2):
            en = alloc_energy(f"en00_{dtp}")
            tanh_pair(ps00[2 * dtp], en, 0, 2 * dtp)
            tanh_pair(ps00[2 * dtp + 1], en, 0, 2 * dtp + 1)
            score_pair(sc00, en, dtp)
        for dtp in range(2, DT // 2):
            en = alloc_energy(f"en00_{dtp}")
            for i in range(2):
                dt = 2 * dtp + i
                ps = ep_ps.tile([128, LC], F32, tag="ep", name=f"ps00b_{dt}")
                for kp in range(KP):
                    nc.tensor.matmul(
                        ps,
                        lhsT=w_pair(kp, dt),
                        rhs=enc_pair(enc00, kp),
                        start=(kp == 0),
                        stop=(kp == KP - 1),
                        perf_mode=PM.DoubleRow,
                    )
                tanh_pair(ps, en, 0, dt)
            score_pair(sc00, en, dtp)

        def kmajor_chunk(b, c, enc_tile):
            """Compute one chunk's scores consuming enc pairs in DMA
            arrival order: dt 0-3 accumulate pair-major across 4 psum
            groups, then dt 4-7 run dt-major at full speed."""
            ps = {
                dt: ep_ps.tile([128, LC], F32, tag="ep", name=f"km_{b}_{c}_{dt}")
                for dt in range(4)
            }
            for kp in range(KP):
                for dt in range(4):
                    nc.tensor.matmul(
                        ps[dt],
                        lhsT=w_pair(kp, dt),
                        rhs=enc_pair(enc_tile, kp),
                        start=(kp == 0),
                        stop=(kp == KP - 1),
                        perf_mode=PM.DoubleRow,
                    )
            sc = sc_ps.tile([1, LC], F32, tag="sc", name=f"km_sc_{b}_{c}")
            for dtp in range(2):
                en = alloc_energy(f"enk_{b}_{c}_{dtp}")
                tanh_pair(ps[2 * dtp], en, b, 2 * dtp)
                tanh_pair(ps[2 * dtp + 1], en, b, 2 * dtp + 1)
                score_pair(sc, en, dtp)
            for dtp in range(2, DT // 2):
                en = alloc_energy(f"enk_{b}_{c}_{dtp}")
                for i in range(2):
                    dt = 2 * dtp + i
                    p2 = ep_ps.tile(
                        [128, LC], F32, tag="ep", name=f"km2_{b}_{c}_{dt}"
                    )
                    for kp in range(KP):
                        nc.tensor.matmul(
                            p2,
                            lhsT=w_pair(kp, dt),
                            rhs=enc_pair(enc_tile, kp),
                            start=(kp == 0),
                            stop=(kp == KP - 1),
                            perf_mode=PM.DoubleRow,
                        )
                    tanh_pair(p2, en, b, dt)
                score_pair(sc, en, dtp)
            return sc

        def dtmajor_chunk(b, c, enc_tile):
            sc = sc_ps.tile([1, LC], F32, tag="sc", name=f"dm_sc_{b}_{c}")
            for dtp in range(DT // 2):
                en = alloc_energy(f"en_{b}_{c}_{dtp}")
                for i in range(2):
                    dt = 2 * dtp + i
                    ps = ep_ps.tile(
                        [128, LC], F32, tag="ep", name=f"dm_{b}_{c}_{dt}"
                    )
                    for kp in range(KP):
                        nc.tensor.matmul(
                            ps,
                            lhsT=w_pair(kp, dt),
                            rhs=enc_pair(enc_tile, kp),
                            start=(kp == 0),
                            stop=(kp == KP - 1),
                            perf_mode=PM.DoubleRow,
                        )
                    tanh_pair(ps, en, b, dt)
                score_pair(sc, en, dtp)
            return sc

        def score_to_wexp(sc, wexp, b, c):
            """Exp the raw chunk scores into the batch's broadcast weight
            tile; returns the per-partition denominator contribution."""
            s_sb = smalls.tile([1, LC], F32R, tag="ssb", name=f"ssb_{b}_{c}")
            nc.scalar.copy(out=s_sb, in_=sc)
            wb = wb_ps.tile([128, LC], F32, tag="wb", name=f"wb_{b}_{c}")
            nc.tensor.matmul(wb, lhsT=ones_sb, rhs=s_sb, start=True, stop=True)
            den_c = smalls.tile([128, 1], F32, tag=f"den{c}", name=f"den_{b}_{c}")
            nc.scalar.activation(
                out=wexp[:, c * LC : (c + 1) * LC],
                in_=wb,
                func=AF.Exp,
                bias=0.0,
                scale=1.0 / VSCALE,
                accum_out=den_c,
            )
            return den_c

        def batch_att(b, encb_tile, wexp):
            """Fused weighted reduce over the full L per k-tile, split
            ~3:1 between DVE (fused STT) and ACT (accum-copy over DVE
            pair-products). Emits the DVE work now; returns a closure
            with the ACT half so the caller can defer it past the next
            chunk's tanh chain (else ACT bunches and PE starves on psum
            banks)."""
            w_pairbc = bass.AP(
                tensor=wexp.tensor,
                offset=wexp.offset,
                ap=[wexp.ap[0], [0, 2], wexp.ap[1]],
            )
            pprods = []
            for kp in range(KT // 2 - 1, KT // 2):  # k 14..15 -> ACT
                k = 2 * kp
                t, kk = (encb_tile[0], k) if k < KH else (encb_tile[1], k - KH)
                prod = prod_pool.tile(
                    [128, 2, L], BF16, tag="prodp", name=f"prp_{b}_{kp}"
                )
                nc.vector.tensor_mul(out=prod, in0=t[:, kk : kk + 2, :], in1=w_pairbc)
                pprods.append((k, prod))
            for k in range(KT - 2):
                col = b * KT + k
                prod = prod_pool.tile([128, L], BF16, tag="prod", name=f"pr_{b}_{k}")
                nc.vector.scalar_tensor_tensor(
                    out=prod,
                    in0=encb_sl(encb_tile, k),
                    scalar=1.0,
                    in1=wexp,
                    op0=ALU.mult,
                    op1=ALU.mult,
                    accum_out=att_all[:, col : col + 1],
                )

            def act_half():
                scr = smalls.tile([128, L], BF16, tag="ascr", name=f"ascr_b{b}")
                for k, prod in pprods:
                    for i in range(2):
                        col = b * KT + k + i
                        nc.scalar.activation(
                            out=scr,
                            in_=prod[:, i, :],
                            func=AF.Copy,
                            bias=0.0,
                            scale=1.0,
                            accum_out=att_all[:, col : col + 1],
                        )

            return act_half

        def chunk_att(b, c, encb_tile, wexp):
            """Per-chunk variant (used for the last batch to keep the
            kernel tail to one chunk's reduce)."""
            atmp = None
            if c > 0:
                atmp = smalls.tile([128, KT], F32, tag="atmp", name=f"atmp_{b}_{c}")
            for k in range(KT):
                col = b * KT + k
                prod = prod_pool.tile(
                    [128, L], BF16, tag="prod", name=f"prc_{b}_{c}_{k}"
                )
                dst = att_all[:, col : col + 1] if c == 0 else atmp[:, k : k + 1]
                nc.vector.scalar_tensor_tensor(
                    out=prod[:, 0:LC],
                    in0=encb_sl(encb_tile, k, c * LC, LC),
                    scalar=1.0,
                    in1=wexp[:, c * LC : (c + 1) * LC],
                    op0=ALU.mult,
                    op1=ALU.mult,
                    accum_out=dst,
                )
            if c > 0:
                cols = slice(b * KT, (b + 1) * KT)
                nc.vector.tensor_add(
                    out=att_all[:, cols], in0=att_all[:, cols], in1=atmp
                )

        def chunk_att_tail(b, c, encb_tile, wexp):
            """Kernel-tail variant: split the reduce between DVE (fused
            STT, k 0-7) and ACT (accum-copy over DVE pair-products,
            k 8-15) so the exposed tail is ~2/3 shorter."""
            atmp = None
            if c > 0:
                atmp = smalls.tile([128, KT], F32, tag="atmp", name=f"atmpt_{b}_{c}")

            def dst(k):
                if c == 0:
                    col = b * KT + k
                    return att_all[:, col : col + 1]
                return atmp[:, k : k + 1]

            w_sl = wexp[:, c * LC : (c + 1) * LC]
            w_pairbc = bass.AP(
                tensor=w_sl.tensor,
                offset=w_sl.offset,
                ap=[w_sl.ap[0], [0, 2], w_sl.ap[1]],
            )
            pprods = []
            for kp in range(KT // 4, KT // 2):
                k = 2 * kp
                t, kk = (encb_tile[0], k) if k < KH else (encb_tile[1], k - KH)
                prod = prod_pool.tile(
                    [128, 2, LC], BF16, tag="prodt", name=f"prp_{b}_{c}_{kp}"
                )
                nc.vector.tensor_mul(
                    out=prod,
                    in0=t[:, kk : kk + 2, c * LC : (c + 1) * LC],
                    in1=w_pairbc,
                )
                pprods.append((k, prod))
            for k in range(KT // 2):
                prod = prod_pool.tile(
                    [128, L], BF16, tag="prod", name=f"prt_{b}_{c}_{k}"
                )
                nc.vector.scalar_tensor_tensor(
                    out=prod[:, 0:LC],
                    in0=encb_sl(encb_tile, k, c * LC, LC),
                    scalar=1.0,
                    in1=w_sl,
                    op0=ALU.mult,
                    op1=ALU.mult,
                    accum_out=dst(k),
                )

            def act_half():
                scr = smalls.tile([128, LC], BF16, tag="ascr", name=f"ascr_{b}_{c}")
                for k, prod in pprods:
                    for i in range(2):
                        nc.scalar.activation(
                            out=scr,
                            in_=prod[:, i, :],
                            func=AF.Copy,
                            bias=0.0,
                            scale=1.0,
                            accum_out=dst(k + i),
                        )
                if c > 0:
                    cols = slice(b * KT, (b + 1) * KT)
                    nc.vector.tensor_add(
                        out=att_all[:, cols], in0=att_all[:, cols], in1=atmp
                    )

            return act_half

        def batch_epilogue(b, dens):
            """Transpose the raw attention columns and store, folding the
            softmax normalization into the ACT psum->sbuf copy (scale)."""
            rden = smalls.tile([128, 1], F32, tag="rden")
            nc.vector.tensor_add(out=rden, in0=dens[0], in1=dens[1])
            for extra in dens[2:]:
                nc.vector.tensor_add(out=rden, in0=rden, in1=extra)
            nc.vector.reciprocal(out=rden, in_=rden)
            cols = slice(b * KT, (b + 1) * KT)
            att_bt = att_ps_pool.tile([KT, 128], F32, tag="abt")
            nc.tensor.transpose(att_bt, att_all[:, cols], eye_sb)
            att_sb = smalls.tile([KT, 128], F32, tag="asb")
            nc.scalar.activation(
                out=att_sb,
                in_=att_bt,
                func=AF.Copy,
                bias=0.0,
                scale=rden[0:KT, :],
            )
            nc.sync.dma_start(
                out=att[b].rearrange("(k p) -> k p", p=128), in_=att_sb
            )

        # ---- main loop (chunk (0,0) scores already computed above) ------
        wexp_b = wexp_pool.tile([128, L], BF16, tag="wexp", name="wexp_0")
        dens = [score_to_wexp(sc00, wexp_b, 0, 0)]
        encb_b = None
        encb_next = None
        pending_act = []
        for b in range(BL):
            last_b = b == BL - 1
            for c in range(NLC):
                if (b, c) == (0, 0):
                    continue
                enc_tile = alloc_chunk8(f"enc_{b}_{c}")
                load_chunk8(b, c, enc_tile)
                if c == 0:
                    wexp_b = wexp_pool.tile(
                        [128, L], BF16, tag="wexp", name=f"wexp_{b}"
                    )
                    if b == 1:
                        # batch 1's bf16 copy loads behind chunk (1,0)
                        encb_next = alloc_encb("encb_1")
                        load_encb(1, encb_next)
                    encb_b = encb_next
                else:
                    if b == 0:
                        # batch 0's bf16 copy loads behind chunk (0,1)
                        encb_b = alloc_encb("encb_0")
                        load_encb(0, encb_b)
                    if 1 <= b < BL - 1:
                        # prefetch next batch's bf16 copy a chunk early
                        encb_next = alloc_encb(f"encb_{b + 1}")
                        load_encb(b + 1, encb_next)
                if (b, c) in ((0, 1), (1, 0), (1, 1)):
                    sc = kmajor_chunk(b, c, enc_tile)
                else:
                    sc = dtmajor_chunk(b, c, enc_tile)
                dens.append(score_to_wexp(sc, wexp_b, b, c))
                # deferred ACT halves of earlier reduces go here, where
                # ACT has inter-chunk slack (after this chunk's scores)
                for fn in pending_act:
                    fn()
                pending_act = []
                if c == 1 and b > 0:
                    # deferred epilogue: PE transpose of batch b-1 sits
                    # a full chunk behind its DVE reduce, so it never
                    # stalls PE waiting on the normalize
                    batch_epilogue(b - 1, prev_dens)
                if b >= BL - 2:
                    # last two batches: per-chunk DVE/ACT split spreads
                    # the late reduce load and keeps the tail short
                    pending_act.append(chunk_att_tail(b, c, encb_b, wexp_b))
            if b < BL - 2:
                pending_act.append(batch_att(b, encb_b, wexp_b))
            prev_dens = dens
            dens = []
        for fn in pending_act:
            fn()
        batch_epilogue(BL - 1, prev_dens)

    return nc


def shard_inputs(dec_hidden, enc_output, W_score, b_score, v):
    """Full inputs -> per-core input maps (host-side layout staging)."""
    dec_hidden = np.ascontiguousarray(dec_hidden, dtype=np.float32)
    W_score = np.asarray(W_score, dtype=np.float32)
    # dec_proj + b_score computed host-side (0.05% of the FLOPs)
    bias_full = dec_hidden @ W_score[:DEC] + np.asarray(b_score, dtype=np.float32)
    # W_enc tiled partition-major: [p, k, d]
    wenc8 = np.ascontiguousarray(
        (W_score[DEC:] * WSCALE).reshape(KT, 128, DEC).transpose(1, 0, 2)
    ).astype(ml_dtypes.float8_e4m3)
    v_pd = np.asarray(v, dtype=np.float32).reshape(DT, 128).T * VSCALE
    v_mat = np.zeros((128, DT, 16), dtype=ml_dtypes.float8_e4m3)
    v_mat[:, :, 0] = v_pd.astype(ml_dtypes.float8_e4m3)
    eye = np.eye(128, dtype=np.float32)

    in_maps = []
    for core in range(N_CORES):
        sl = slice(core * BL, (core + 1) * BL)
        # (L, BL, 2E) -> (BL, 2E, L)
        enc_t = np.ascontiguousarray(
            np.asarray(enc_output[:, sl, :], dtype=np.float32).transpose(1, 2, 0)
        )
        # fp8 chunk-major partition-tiled: [b, c, half, p, k, l]
        enc8_t = np.ascontiguousarray(
            enc_t.reshape(BL, 2, KH, 128, NLC, LC).transpose(0, 4, 1, 3, 2, 5)
        ).astype(ml_dtypes.float8_e4m3)
        # bf16 full-L partition-tiled: [b, half, p, k, l]
        encb_t = np.ascontiguousarray(
            enc_t.reshape(BL, 2, KH, 128, L).transpose(0, 1, 3, 2, 4)
        ).astype(ml_dtypes.bfloat16)
        # (BL, DEC) -> [p, dt, b]
        bias_kpb = np.ascontiguousarray(
            bias_full[sl].T.reshape(DT, 128, BL).transpose(1, 0, 2)
        )
        in_maps.append(
            {
                "enc8_t": enc8_t,
                "encb_t": encb_t,
                "ones": np.ones((1, 128), dtype=np.float32),
                "bias_kpb": bias_kpb,
                "wenc8": wenc8,
                "v_mat": v_mat,
                "eye": eye,
            }
        )
    return in_maps


_NC_CACHE = None


def kernel(dec_hidden, enc_output, W_score, b_score, v):
    global _NC_CACHE
    if _NC_CACHE is None:
        _NC_CACHE = build_nc()
    nc = _NC_CACHE
    in_maps = shard_inputs(dec_hidden, enc_output, W_score, b_score, v)
    res = run_bass_kernel_spmd(nc, in_maps, list(range(N_CORES)))
    return np.concatenate([res.results[i]["att"] for i in range(N_CORES)], axis=0)


# revision 39
# speedup vs baseline: 1.2273x; 1.0399x over previous
"""Bahdanau additive-attention kernel for Trainium2, data-parallel over
batch across 8 NeuronCores.

Per batch b:
    energy  = tanh(dec_proj[b] + enc[b] @ W_enc + b_score)   # (L, DEC)
    scores  = energy @ v                                     # (L,)
    alpha   = softmax(scores)
    att[b]  = alpha @ enc[b]                                 # (2E,)

On-device layout (per core, 8 batches):
  - enc is staged host-side TWICE, both partition-major-tiled so DMA
    descriptors are 4-16KB runs: enc8_t (fp8e4m3, chunk-major) feeds the
    PE energy matmul in perf_mode=DoubleRow (two k-subtiles per
    instruction, measured ~2x over bf16/f32r at N=512); encb_t (bf16,
    full-L tiles) feeds the DVE attention reduce. W_enc is pre-scaled by
    128 on the host so its values sit mid-range in e4m3; the tanh
    activation rescales the psum by 1/128.
  - dec_proj preamble runs in bf16 (error negligible vs fp8 energy).
  - scores = v . energyT via PE matvec in bf16 over d-tiles.
  - softmax skips the max-subtraction: |scores| <= sum|v| = 32, safely
    inside the fp32 exp range. Raw scores broadcast to 128 partitions
    with a K=1 ones-matmul; Exp runs on the broadcast tile (bf16 out)
    with accum_out giving the replicated denominator per partition.
  - att^T accumulates via fused scalar_tensor_tensor on DVE:
    accum_out[e,1] = sum_l enc_bf16[e,l] * exp_scores[l], one full-L
    instruction per (batch, k-tile) to amortize DVE fixed overhead. The
    last batch runs per-chunk so only one chunk's reduce sits in the
    kernel tail.
  - startup is DMA-paced, so batch 0 chunk 0 consumes enc in half-tile
    arrival order with the dec_proj preamble matmuls behind it.
"""

import numpy as np
import ml_dtypes
from contextlib import ExitStack

import concourse.bass as bass
import concourse.tile as tile
from concourse import mybir
from concourse.bass_utils import run_bass_kernel_spmd
from concourse.vector_clock import ScopedClock, VectorClock

N_CORES = 8
B, L, DEC, ENC2 = 64, 1024, 1024, 2048
BL = B // N_CORES  # batches per core
KT = ENC2 // 128   # contraction tiles over e
KP = KT // 2       # DoubleRow pairs over e
KH = KT // 2       # k-tiles per half (SBUF tile granularity)
DT = DEC // 128    # d tiles
LC = 512           # l-chunk (one PSUM bank of f32)
NLC = L // LC
WSCALE = 128.0     # host-side W_enc scaling for fp8 range
VSCALE = 32.0      # host-side v scaling for fp8 range

F32 = mybir.dt.float32
F32R = mybir.dt.float32r
F8 = mybir.dt.float8e4
BF16 = mybir.dt.bfloat16
AF = mybir.ActivationFunctionType
ALU = mybir.AluOpType
PM = mybir.MatmulPerfMode


def _patch_tile_drain():
    """Workarounds for this container's walrus build.

    1. The Tile tail drain carries one sem wait per touched proc; walrus
       rejects >2 on the CTRL encoding. Split the waits onto single-wait
       SP nops (SP executes in order, so the drain then needs none).
    2. Any instruction with 2+ sem waits can fail codegen (the matmul
       LW encoding holds a single wait). Split multi-wait instructions:
       excess waits move onto same-engine InstNoOp carriers inserted
       just before; engine program order makes this equivalent.
    """
    if getattr(tile.TileContext, "_drain_patched", False):
        return

    def _drain_and_barrier(self, tick_clock, wait_clock):
        vec = list(tick_clock.global_clock)
        n = len(vec)
        for i in range(n):
            if vec[i] <= 0:
                continue
            part = [0] * n
            part[i] = vec[i]
            nop_inst = self.nc.sync.nop(nofuse=True)
            wait_clock.add_sem_waits(
                nop_inst.ins, ScopedClock({None: VectorClock(part)})
            )
        self.nc.sync.drain()
        self.nc.all_engine_barrier()
        assert self.sems is not None
        popped = self.nc._tile_sem_poison_stack.pop()
        assert popped is self._sem_poison
        self.nc.clear_and_free_semaphores(list(self.sems.allocated().values()))
        self.nc.all_engine_barrier()

    tile.TileContext._drain_and_barrier = _drain_and_barrier

    import bass_rust

    orig_lower = tile.TileContext._lower_ordered_insts

    def _lower_with_wait_split(self, ordered):
        for insts in ordered.values():
            expanded = []
            for inst in insts:
                si = inst.sync_info
                waits = list(si.on_wait) if si and si.on_wait else []
                if len(waits) > 1:
                    for w in waits[:-1]:
                        nop = mybir.InstNoOp(
                            name=self.nc.get_next_instruction_name(),
                            engine=inst.engine,
                            bass_nofuse=True,
                            sync_info=bass_rust.SyncInfo(on_wait=[w], on_update=[]),
                        )
                        self.nc.register_instruction(nop)
                        expanded.append(nop)
                    inst.sync_info = bass_rust.SyncInfo(
                        on_wait=[waits[-1]],
                        on_update=list(si.on_update) if si.on_update else [],
                    )
                expanded.append(inst)
            insts[:] = expanded
        return orig_lower(self, ordered)

    tile.TileContext._lower_ordered_insts = _lower_with_wait_split
    tile.TileContext._drain_patched = True


def build_nc():
    _patch_tile_drain()
    nc = bass.Bass()
    # partition-major tiled layouts (see shard_inputs)
    enc8_t = nc.declare_dram_parameter(
        "enc8_t", [BL, NLC, 2, 128, KH, LC], F8, isOutput=False
    )
    encb_t = nc.declare_dram_parameter(
        "encb_t", [BL, 2, 128, KH, L], BF16, isOutput=False
    )
    wenc8_d = nc.declare_dram_parameter(
        "wenc8", [128, KT, DEC], F8, isOutput=False
    )
    bias_d = nc.declare_dram_parameter("bias_kpb", [128, DT, BL], F32, isOutput=False)
    v_mat = nc.declare_dram_parameter("v_mat", [128, DT, 16], F8, isOutput=False)
    eye = nc.declare_dram_parameter("eye", [128, 128], F32, isOutput=False)
    ones = nc.declare_dram_parameter("ones", [1, 128], F32R, isOutput=False)
    att = nc.declare_dram_parameter("att", [BL, ENC2], F32, isOutput=True)

    with tile.TileContext(nc) as tc, ExitStack() as ctx:
        singles = ctx.enter_context(tc.tile_pool(name="singles", bufs=1))
        smalls = ctx.enter_context(tc.tile_pool(name="smalls", bufs=2))
        enc8_pool = ctx.enter_context(tc.tile_pool(name="enc8", bufs=8))
        encb_pool = ctx.enter_context(tc.tile_pool(name="encb", bufs=6))
        energy_pool = ctx.enter_context(tc.tile_pool(name="energy", bufs=3))
        wexp_pool = ctx.enter_context(tc.tile_pool(name="wexp", bufs=2))
        prod_pool = ctx.enter_context(tc.tile_pool(name="prod", bufs=2))
        ep_ps = ctx.enter_context(tc.tile_pool(name="ep_ps", bufs=4, space="PSUM"))
        sc_ps = ctx.enter_context(tc.tile_pool(name="sc_ps", bufs=2, space="PSUM"))
        wb_ps = ctx.enter_context(tc.tile_pool(name="wb_ps", bufs=1, space="PSUM"))
        att_ps_pool = ctx.enter_context(
            tc.tile_pool(name="att_ps", bufs=1, space="PSUM")
        )

        # ---- persistent tiles -------------------------------------------
        wenc = singles.tile([128, KT, DEC], F8)  # W_enc*128, (e-tile, k) x d
        v_sb = singles.tile([128, DT, 16], F8)
        eye_sb = singles.tile([128, 128], F32)
        bias_sb = singles.tile([128, DT, BL], F32)  # dec_proj + b_score
        att_all = singles.tile([128, KT * BL], F32)  # att^T cols = b*KT+k
        ones_sb = singles.tile([1, 128], F32R)

        def alloc_chunk8(nm):
            a = enc8_pool.tile([128, KH, LC], F8, tag="enc", name=f"{nm}a")
            bb = enc8_pool.tile([128, KH, LC], F8, tag="enc", name=f"{nm}b")
            return (a, bb)

        def load_chunk8(b, c, ch):
            for h in range(2):
                nc.sync.dma_start(out=ch[h], in_=enc8_t[b, c, h])

        def alloc_encb(nm):
            a = encb_pool.tile([128, KH, L], BF16, tag="encb", name=f"{nm}a")
            bb = encb_pool.tile([128, KH, L], BF16, tag="encb", name=f"{nm}b")
            return (a, bb)

        def load_encb_parts(b, ch, parts):
            for part in parts:
                h, q = part // 2, (part % 2) * (KH // 2)
                nc.sync.dma_start(
                    out=ch[h][:, q : q + KH // 2, :],
                    in_=encb_t[b, h][:, q : q + KH // 2, :],
                )

        def enc_pair(ch, kp):
            """[128, 2, LC] DoubleRow rhs slice for pair (2kp, 2kp+1)."""
            k = 2 * kp
            t, kk = (ch[0], k) if k < KH else (ch[1], k - KH)
            return t[:, kk : kk + 2, :]

        def encb_sl(ch, k, lo=0, width=L):
            t, kk = (ch[0], k) if k < KH else (ch[1], k - KH)
            return t[:, kk, lo : lo + width]

        def w_pair(kp, dt):
            """[128, 2, 128] DoubleRow lhsT slice."""
            return wenc[:, 2 * kp : 2 * kp + 2, dt * 128 : (dt + 1) * 128]

        # ---- startup DMA, in data-arrival order -------------------------
        enc00 = alloc_chunk8("enc00")
        nc.sync.dma_start(out=enc00[0][:, 0:2, :], in_=enc8_t[0, 0, 0][:, 0:2, :])
        nc.sync.dma_start(out=wenc[:, 0:2, :], in_=wenc8_d[:, 0:2, :])
        nc.sync.dma_start(out=enc00[0][:, 2:4, :], in_=enc8_t[0, 0, 0][:, 2:4, :])
        nc.sync.dma_start(out=wenc[:, 2:4, :], in_=wenc8_d[:, 2:4, :])
        nc.sync.dma_start(out=enc00[0][:, 4:8, :], in_=enc8_t[0, 0, 0][:, 4:8, :])
        nc.sync.dma_start(out=wenc[:, 4:8, :], in_=wenc8_d[:, 4:8, :])
        nc.sync.dma_start(out=bias_sb, in_=bias_d[:, :, :])
        nc.sync.dma_start(out=v_sb, in_=v_mat[:, :, :])
        nc.sync.dma_start(out=enc00[1], in_=enc8_t[0, 0, 1])
        nc.sync.dma_start(out=wenc[:, 8:12, :], in_=wenc8_d[:, 8:12, :])
        nc.sync.dma_start(out=wenc[:, 12:16, :], in_=wenc8_d[:, 12:16, :])
        nc.sync.dma_start(out=eye_sb, in_=eye[:, :])
        nc.sync.dma_start(out=ones_sb, in_=ones[:, :])

        # ---- chunk (0,0): consume pairs in half-arrival order ----------
        ps00 = {
            dt: ep_ps.tile([128, LC], F32, tag="ep", name=f"ps00_{dt}")
            for dt in range(4)
        }
        for kp in range(KP):
            for dt in range(4):
                nc.tensor.matmul(
                    ps00[dt],
                    lhsT=w_pair(kp, dt),
                    rhs=enc_pair(enc00, kp),
                    start=(kp == 0),
                    stop=(kp == KP - 1),
                    perf_mode=PM.DoubleRow,
                )

        def tanh_pair(ps, energy, b, dt):
            """tanh one d-tile's psum into half of an fp8 pair tile."""
            nc.scalar.activation(
                out=energy[:, dt % 2, :],
                in_=ps,
                func=AF.Tanh,
                bias=bias_sb[:, dt, b : b + 1],
                scale=1.0 / WSCALE,
            )

        def score_pair(sc, energy, dtp):
            """DoubleRow matvec over one d-tile pair: sc += v . energy."""
            nc.tensor.matmul(
                sc,
                lhsT=v_sb[:, 2 * dtp : 2 * dtp + 2, 0:1],
                rhs=energy,
                start=(dtp == 0),
                stop=(dtp == DT // 2 - 1),
                perf_mode=PM.DoubleRow,
            )

        def alloc_energy(nm):
            return energy_pool.tile([128, 2, LC], F8, tag="energy", name=nm)

        sc00 = sc_ps.tile([1, LC], F32, tag="sc")
        for dtp in range(2):
            en = alloc_energy(f"en00_{dtp}")
            tanh_pair(ps00[2 * dtp], en, 0, 2 * dtp)
            tanh_pair(ps00[2 * dtp + 1], en, 0, 2 * dtp + 1)
            score_pair(sc00, en, dtp)
        for dtp in range(2, DT // 2):
            en = alloc_energy(f"en00_{dtp}")
            for i in range(2):
                dt = 2 * dtp + i
                ps = ep_ps.tile([128, LC], F32, tag="ep", name=f"ps00b_{dt}")
                for kp in range(KP):
                    nc.tensor.matmul(
                        ps,
                        lhsT=w_pair(kp, dt),
                        rhs=enc_pair(enc00, kp),
                        start=(kp == 0),
                        stop=(kp == KP - 1),
                        perf_mode=PM.DoubleRow,
                    )
                tanh_pair(ps, en, 0, dt)
            score_pair(sc00, en, dtp)

        def make_chunk(b, c, enc_tile):
            """Return (sc, group emitters, matvec emitters) for one
            chunk. The caller interleaves them with the previous chunk's
            tail ops (software pipelining) so ACT latencies never sit
            exposed in the in-order PE stream."""
            sc = sc_ps.tile([1, LC], F32, tag="sc", name=f"sc_{b}_{c}")
            ens = {}

            def make_group(dtp):
                def emit():
                    en = alloc_energy(f"en_{b}_{c}_{dtp}")
                    for i in range(2):
                        dt = 2 * dtp + i
                        ps = ep_ps.tile(
                            [128, LC], F32, tag="ep", name=f"dm_{b}_{c}_{dt}"
                        )
                        for kp in range(KP):
                            nc.tensor.matmul(
                                ps,
                                lhsT=w_pair(kp, dt),
                                rhs=enc_pair(enc_tile, kp),
                                start=(kp == 0),
                                stop=(kp == KP - 1),
                                perf_mode=PM.DoubleRow,
                            )
                        tanh_pair(ps, en, b, dt)
                    ens[dtp] = en

                return emit

            def make_mv(dtp):
                def emit():
                    score_pair(sc, ens[dtp], dtp)

                return emit

            groups = [make_group(p) for p in range(DT // 2)]
            mvs = [make_mv(p) for p in range(DT // 2)]
            return sc, groups, mvs

        def score_to_wexp(sc, wexp, b, c):
            """Exp the raw chunk scores into the batch's broadcast weight
            tile; returns the per-partition denominator contribution."""
            s_sb = smalls.tile([1, LC], F32R, tag="ssb", name=f"ssb_{b}_{c}")
            nc.scalar.copy(out=s_sb, in_=sc)
            wb = wb_ps.tile([128, LC], F32, tag="wb", name=f"wb_{b}_{c}")
            nc.tensor.matmul(wb, lhsT=ones_sb, rhs=s_sb, start=True, stop=True)
            den_c = smalls.tile([128, 1], F32, tag=f"den{c}", name=f"den_{b}_{c}")
            nc.scalar.activation(
                out=wexp[:, c * LC : (c + 1) * LC],
                in_=wb,
                func=AF.Exp,
                bias=0.0,
                scale=1.0 / VSCALE,
                accum_out=den_c,
            )
            return den_c

        def batch_att(b, encb_tile, wexp):
            """Fused weighted reduce over the full L per k-tile, split
            ~3:1 between DVE (fused STT) and ACT (accum-copy over DVE
            pair-products). Emits the DVE work now; returns a closure
            with the ACT half so the caller can defer it past the next
            chunk's tanh chain (else ACT bunches and PE starves on psum
            banks)."""
            w_pairbc = bass.AP(
                tensor=wexp.tensor,
                offset=wexp.offset,
                ap=[wexp.ap[0], [0, 2], wexp.ap[1]],
            )
            pprods = []
            for kp in range(KT // 2 - 1, KT // 2):  # k 14..15 -> ACT
                k = 2 * kp
                t, kk = (encb_tile[0], k) if k < KH else (encb_tile[1], k - KH)
                prod = prod_pool.tile(
                    [128, 2, L], BF16, tag="prodp", name=f"prp_{b}_{kp}"
                )
                nc.vector.tensor_mul(out=prod, in0=t[:, kk : kk + 2, :], in1=w_pairbc)
                pprods.append((k, prod))
            for k in range(KT - 2):
                col = b * KT + k
                prod = prod_pool.tile([128, L], BF16, tag="prod", name=f"pr_{b}_{k}")
                nc.vector.scalar_tensor_tensor(
                    out=prod,
                    in0=encb_sl(encb_tile, k),
                    scalar=1.0,
                    in1=wexp,
                    op0=ALU.mult,
                    op1=ALU.mult,
                    accum_out=att_all[:, col : col + 1],
                )

            def act_half():
                scr = smalls.tile([128, L], BF16, tag="ascr", name=f"ascr_b{b}")
                for k, prod in pprods:
                    for i in range(2):
                        col = b * KT + k + i
                        nc.scalar.activation(
                            out=scr,
                            in_=prod[:, i, :],
                            func=AF.Copy,
                            bias=0.0,
                            scale=1.0,
                            accum_out=att_all[:, col : col + 1],
                        )

            return act_half

        def chunk_att_tail(b, c, encb_tile, wexp):
            """Kernel-tail variant: split the reduce between DVE (fused
            STT, k 0-7) and ACT (accum-copy over DVE pair-products,
            k 8-15) so the exposed tail is ~2/3 shorter."""
            atmp = None
            if c > 0:
                atmp = smalls.tile([128, KT], F32, tag="atmp", name=f"atmpt_{b}_{c}")

            def dst(k):
                if c == 0:
                    col = b * KT + k
                    return att_all[:, col : col + 1]
                return atmp[:, k : k + 1]

            w_sl = wexp[:, c * LC : (c + 1) * LC]
            w_pairbc = bass.AP(
                tensor=w_sl.tensor,
                offset=w_sl.offset,
                ap=[w_sl.ap[0], [0, 2], w_sl.ap[1]],
            )
            pprods = []
            for kp in range(KT // 4, KT // 2):
                k = 2 * kp
                t, kk = (encb_tile[0], k) if k < KH else (encb_tile[1], k - KH)
                prod = prod_pool.tile(
                    [128, 2, LC], BF16, tag="prodt", name=f"prp_{b}_{c}_{kp}"
                )
                nc.vector.tensor_mul(
                    out=prod,
                    in0=t[:, kk : kk + 2, c * LC : (c + 1) * LC],
                    in1=w_pairbc,
                )
                pprods.append((k, prod))
            for k in range(KT // 2):
                prod = prod_pool.tile(
                    [128, L], BF16, tag="prod", name=f"prt_{b}_{c}_{k}"
                )
                nc.vector.scalar_tensor_tensor(
                    out=prod[:, 0:LC],
                    in0=encb_sl(encb_tile, k, c * LC, LC),
                    scalar=1.0,
                    in1=w_sl,
                    op0=ALU.mult,
                    op1=ALU.mult,
                    accum_out=dst(k),
                )

            def act_half():
                scr = smalls.tile([128, LC], BF16, tag="ascr", name=f"ascr_{b}_{c}")
                for k, prod in pprods:
                    for i in range(2):
                        nc.scalar.activation(
                            out=scr,
                            in_=prod[:, i, :],
                            func=AF.Copy,
                            bias=0.0,
                            scale=1.0,
                            accum_out=dst(k + i),
                        )
                if c > 0:
                    cols = slice(b * KT, (b + 1) * KT)
                    nc.vector.tensor_add(
                        out=att_all[:, cols], in0=att_all[:, cols], in1=atmp
                    )

            return act_half

        def batch_epilogue(b, dens):
            """Transpose the raw attention columns and store, folding the
            softmax normalization into the ACT psum->sbuf copy (scale)."""
            rden = smalls.tile([128, 1], F32, tag="rden")
            nc.vector.tensor_add(out=rden, in0=dens[0], in1=dens[1])
            for extra in dens[2:]:
                nc.vector.tensor_add(out=rden, in0=rden, in1=extra)
            nc.vector.reciprocal(out=rden, in_=rden)
            cols = slice(b * KT, (b + 1) * KT)
            att_bt = att_ps_pool.tile([KT, 128], F32, tag="abt")
            nc.tensor.transpose(att_bt, att_all[:, cols], eye_sb)
            att_sb = smalls.tile([KT, 128], F32, tag="asb")
            nc.scalar.activation(
                out=att_sb,
                in_=att_bt,
                func=AF.Copy,
                bias=0.0,
                scale=rden[0:KT, :],
            )
            nc.sync.dma_start(
                out=att[b].rearrange("(k p) -> k p", p=128), in_=att_sb
            )

        # ---- main loop: 1-chunk software pipeline -----------------------
        # Emission per iteration: [g0, prev.mv3, g1, prev bookkeeping
        # (scores->exp, reduces, epilogues), mv0, g2, mv1, g3, mv2] so
        # the previous chunk's ACT-latency tail hides behind this
        # chunk's matmul groups in the in-order PE stream.
        wexp_map = {}
        encb_map = {}
        state = {"dens": [], "prev_dens": None, "pending": []}

        def bookkeeping(pb, pc, sc):
            if pc == 0:
                wexp_map[pb] = wexp_pool.tile(
                    [128, L], BF16, tag="wexp", name=f"wexp_{pb}"
                )
                state["dens"] = []
            wexp = wexp_map[pb]
            state["dens"].append(score_to_wexp(sc, wexp, pb, pc))
            for fn in state["pending"]:
                fn()
            state["pending"] = []
            if pc == 1 and pb > 0:
                batch_epilogue(pb - 1, state["prev_dens"])
            if pb >= BL - 2:
                state["pending"].append(
                    chunk_att_tail(pb, pc, encb_map[pb], wexp)
                )
            elif pc == 1:
                state["pending"].append(batch_att(pb, encb_map[pb], wexp))
            if pc == 1:
                state["prev_dens"] = state["dens"]

        prev_bk = (0, 0, sc00)
        prev_mv3 = None
        for b in range(BL):
            for c in range(NLC):
                if (b, c) == (0, 0):
                    continue
                enc_tile = alloc_chunk8(f"enc_{b}_{c}")
                load_chunk8(b, c, enc_tile)
                # bf16-copy prefetch, spread across chunk slots
                if (b, c) == (0, 1):
                    encb_map[0] = alloc_encb("encb_0")
                    load_encb_parts(0, encb_map[0], [0, 1, 2, 3])
                elif (b, c) == (1, 0):
                    encb_map[1] = alloc_encb("encb_1")
                    load_encb_parts(1, encb_map[1], [0, 1])
                elif (b, c) == (1, 1):
                    load_encb_parts(1, encb_map[1], [2, 3])
                    encb_map[2] = alloc_encb("encb_2")
                    load_encb_parts(2, encb_map[2], [0, 1])
                elif c == 0 and b >= 2:
                    load_encb_parts(b, encb_map[b], [2, 3])
                elif c == 1 and 2 <= b < BL - 1:
                    encb_map[b + 1] = alloc_encb(f"encb_{b + 1}")
                    load_encb_parts(b + 1, encb_map[b + 1], [0, 1])
                sc, groups, mvs = make_chunk(b, c, enc_tile)
                groups[0]()
                if prev_mv3 is not None:
                    prev_mv3()
                groups[1]()
                bookkeeping(*prev_bk)
                mvs[0]()
                groups[2]()
                mvs[1]()
                groups[3]()
                mvs[2]()
                prev_bk = (b, c, sc)
                prev_mv3 = mvs[3]
        prev_mv3()
        bookkeeping(*prev_bk)
        for fn in state["pending"]:
            fn()
        batch_epilogue(BL - 1, state["prev_dens"])

    return nc


def shard_inputs(dec_hidden, enc_output, W_score, b_score, v):
    """Full inputs -> per-core input maps (host-side layout staging)."""
    dec_hidden = np.ascontiguousarray(dec_hidden, dtype=np.float32)
    W_score = np.asarray(W_score, dtype=np.float32)
    # dec_proj + b_score computed host-side (0.05% of the FLOPs)
    bias_full = dec_hidden @ W_score[:DEC] + np.asarray(b_score, dtype=np.float32)
    # W_enc tiled partition-major: [p, k, d]
    wenc8 = np.ascontiguousarray(
        (W_score[DEC:] * WSCALE).reshape(KT, 128, DEC).transpose(1, 0, 2)
    ).astype(ml_dtypes.float8_e4m3)
    v_pd = np.asarray(v, dtype=np.float32).reshape(DT, 128).T * VSCALE
    v_mat = np.zeros((128, DT, 16), dtype=ml_dtypes.float8_e4m3)
    v_mat[:, :, 0] = v_pd.astype(ml_dtypes.float8_e4m3)
    eye = np.eye(128, dtype=np.float32)

    in_maps = []
    for core in range(N_CORES):
        sl = slice(core * BL, (core + 1) * BL)
        # (L, BL, 2E) -> (BL, 2E, L)
        enc_t = np.ascontiguousarray(
            np.asarray(enc_output[:, sl, :], dtype=np.float32).transpose(1, 2, 0)
        )
        # fp8 chunk-major partition-tiled: [b, c, half, p, k, l]
        enc8_t = np.ascontiguousarray(
            enc_t.reshape(BL, 2, KH, 128, NLC, LC).transpose(0, 4, 1, 3, 2, 5)
        ).astype(ml_dtypes.float8_e4m3)
        # bf16 full-L partition-tiled: [b, half, p, k, l]
        encb_t = np.ascontiguousarray(
            enc_t.reshape(BL, 2, KH, 128, L).transpose(0, 1, 3, 2, 4)
        ).astype(ml_dtypes.bfloat16)
        # (BL, DEC) -> [p, dt, b]
        bias_kpb = np.ascontiguousarray(
            bias_full[sl].T.reshape(DT, 128, BL).transpose(1, 0, 2)
        )
        in_maps.append(
            {
                "enc8_t": enc8_t,
                "encb_t": encb_t,
                "ones": np.ones((1, 128), dtype=np.float32),
                "bias_kpb": bias_kpb,
                "wenc8": wenc8,
                "v_mat": v_mat,
                "eye": eye,
            }
        )
    return in_maps


_NC_CACHE = None


def kernel(dec_hidden, enc_output, W_score, b_score, v):
    global _NC_CACHE
    if _NC_CACHE is None:
        _NC_CACHE = build_nc()
    nc = _NC_CACHE
    in_maps = shard_inputs(dec_hidden, enc_output, W_score, b_score, v)
    res = run_bass_kernel_spmd(nc, in_maps, list(range(N_CORES)))
    return np.concatenate([res.results[i]["att"] for i in range(N_CORES)], axis=0)
